# revision 1
# baseline (speedup 1.0000x reference)
"""RWKV-style Block kernel for 8 Trainium2 NeuronCores (batch-parallel SPMD).

Strategy:
  - Host pre-computes: transposed bf16 weights (with ln affine scale folded in),
    per-channel constants (decay lambda, e^u, mixes, biases, t=0 fixups).
  - Each core processes one batch element b fully on-device:
      ln1 stats -> hn (normalized, bf16) -> DMA-transpose -> hnT
      time-mixes (tensor_scalar ops in transposed land)
      k/v/r matmuls (bf16, PSUM f32)
      WKV via linear recurrence using tensor_tensor_scan (fp32)
      att/short matmuls -> out1 -> ln2 -> gn -> FFN (relu^2 MLP) -> out
  - Everything streamed through SBUF in chunks, large intermediates spilled to
    DRAM scratch (hnT, rwkvT, srT, kk^2, out1).
"""
import numpy as np
import ml_dtypes

import concourse.bass as bass
import concourse.bacc as bacc
import concourse.mybir as mybir
import concourse.tile as tile
from concourse.bass_utils import run_bass_kernel_spmd

F32 = mybir.dt.float32
BF16 = mybir.dt.bfloat16
AL = mybir.AluOpType
ACT = mybir.ActivationFunctionType
BF = ml_dtypes.bfloat16

B, C, F = 8, 1024, 4096
P = 128
CT = C // P          # 8 c-tiles
FT = F // P          # 32 f-tiles
NC2 = C // 512       # 2
EPS = 1e-5

# cvec slots
S_LAM, S_EU, S_MK, S_MV, S_MR, S_KB, S_VB, S_RB, S_FK, S_FV, S_FR, S_FFK, S_FFR, S_RRB, S_FRR = range(15)
NSLOT = 15


def _bcast_free(col_ap, n):
    """per-partition [128,1] column AP -> [128,n] stride-0 broadcast AP."""
    return bass.AP(tensor=col_ap.tensor, offset=col_ap.offset,
                   ap=[col_ap.ap[0], [0, n]])


def build_nc(T):
    NJ = T // 512        # big chunks
    TT = T // 128        # t-tiles
    nc = bacc.Bacc("TRN2", target_bir_lowering=False)

    # ---------------- DRAM I/O ----------------
    x_d = nc.dram_tensor("x", [T, C], F32, kind="ExternalInput")
    wkT_d = nc.dram_tensor("wkT", [C, C], BF16, kind="ExternalInput")
    wvT_d = nc.dram_tensor("wvT", [C, C], BF16, kind="ExternalInput")
    wrT_d = nc.dram_tensor("wrT", [C, C], BF16, kind="ExternalInput")
    woT_d = nc.dram_tensor("woT", [C, C], BF16, kind="ExternalInput")
    shT_d = nc.dram_tensor("shT", [C, C], BF16, kind="ExternalInput")
    fwkT_d = nc.dram_tensor("fwkT", [C, F], BF16, kind="ExternalInput")
    fwrT_d = nc.dram_tensor("fwrT", [C, C], BF16, kind="ExternalInput")
    fwvT_d = nc.dram_tensor("fwvT", [F, C], BF16, kind="ExternalInput")
    cvec_d = nc.dram_tensor("cvec", [P, NSLOT, CT], F32, kind="ExternalInput")
    fvec_d = nc.dram_tensor("fvec", [P, 2, FT], F32, kind="ExternalInput")
    srow_d = nc.dram_tensor("srow", [1, C], BF16, kind="ExternalInput")
    out_d = nc.dram_tensor("out", [T, C], F32, kind="ExternalOutput")

    # DRAM scratch
    hnTd = nc.dram_tensor("hnTd", [C, T], BF16)
    rwTd = nc.dram_tensor("rwTd", [C, T], BF16)
    srTd = nc.dram_tensor("srTd", [C, T], BF16)
    kk2d = nc.dram_tensor("kk2d", [F, T], BF16)
    o1d = nc.dram_tensor("o1d", [T, C], F32)

    with tile.TileContext(nc) as tc:
        with tc.tile_pool(name="pool", bufs=1) as pl, \
             tc.tile_pool(name="psum", bufs=2, space="PSUM") as pp:

            # ---- constants ----
            cv = pl.tile([P, NSLOT, CT], F32, tag="cv")
            nc.sync.dma_start(out=cv, in_=cvec_d[:, :, :])
            fv = pl.tile([P, 2, FT], F32, tag="fv")
            nc.sync.dma_start(out=fv, in_=fvec_d[:, :, :])
            srow_bc = pl.tile([P, C], BF16, tag="srow")
            s_ap = srow_d[0:1, :]
            nc.sync.dma_start(out=srow_bc, in_=bass.AP(
                tensor=s_ap.tensor, offset=s_ap.offset, ap=[[0, P], s_ap.ap[1]]))

            def cvc(slot, ci):
                return cv[:, slot, ci:ci + 1]

            musd = pl.tile([P, 2, TT], F32, tag="musd")
            carA = pl.tile([P, CT], F32, tag="carA")
            carB = pl.tile([P, CT], F32, tag="carB")
            hcar = pl.tile([P, CT, 1], BF16, tag="hcar")
            gcar = pl.tile([P, CT, 1], BF16, tag="gcar")
            epst = pl.tile([P, 1], F32, tag="epst")
            nc.vector.memset(epst, EPS)

            # ---- weights: phase 1 ----
            wk_sb = pl.tile([P, CT, C], BF16, tag="w2m", bufs=3)
            nc.sync.dma_start(out=wk_sb, in_=wkT_d[:, :].rearrange("(ci p) co -> p ci co", p=P))
            wv_sb = pl.tile([P, CT, C], BF16, tag="w2m", bufs=3)
            nc.sync.dma_start(out=wv_sb, in_=wvT_d[:, :].rearrange("(ci p) co -> p ci co", p=P))
            wr_sb = pl.tile([P, CT, C], BF16, tag="w2m", bufs=3)
            nc.sync.dma_start(out=wr_sb, in_=wrT_d[:, :].rearrange("(ci p) co -> p ci co", p=P))

            # ================= Phase TM =================
            hn_prev = None
            for j in range(NJ):
                hnc = pl.tile([P, CT, 513], BF16, tag="hnT", bufs=2, name=f"hnc{j}")
                if j == 0:
                    nc.vector.memset(hnc[:, :, 0:1], 0.0)
                else:
                    nc.vector.tensor_copy(hnc[:, :, 0:1], hcar[:, :, :])
                for tl in range(4):
                    tt = 4 * j + tl
                    xt_ = pl.tile([P, C], F32, tag="xin", bufs=2, name=f"xt{tt}")
                    nc.sync.dma_start(out=xt_, in_=x_d[tt * P:(tt + 1) * P, :])
                    st_ = pl.tile([P, 2, 6], F32, tag="st", bufs=2, name=f"st{tt}")
                    nc.vector.bn_stats(out=st_[:, 0, :], in_=xt_[:, 0:512])
                    nc.vector.bn_stats(out=st_[:, 1, :], in_=xt_[:, 512:1024])
                    mv_ = pl.tile([P, 2], F32, tag="mv", bufs=2, name=f"mv{tt}")
                    nc.vector.bn_aggr(out=mv_, in_=st_)
                    nc.vector.tensor_copy(musd[:, 0, tt:tt + 1], mv_[:, 0:1])
                    nc.scalar.activation(musd[:, 1, tt:tt + 1], mv_[:, 1:2], ACT.Sqrt, bias=epst)
                    rstd = pl.tile([P, 1], F32, tag="rstd", bufs=2, name=f"rstd{tt}")
                    nc.vector.reciprocal(rstd, musd[:, 1, tt:tt + 1])
                    hnb = pl.tile([P, C], BF16, tag="hnn", bufs=2, name=f"hnb{tt}")
                    nc.vector.tensor_scalar(hnb, xt_, mv_[:, 0:1], rstd, AL.subtract, AL.mult)
                    for ci in range(CT):
                        trs = pl.tile([P, P], BF16, tag="trs", bufs=3, name=f"trs{tt}_{ci}")
                        nc.sync.dma_start(out=trs, in_=hnb[:, ci * P:(ci + 1) * P], transpose=True)
                        nc.gpsimd.tensor_copy(hnc[:, ci, 1 + tl * P:1 + (tl + 1) * P], trs)
                # carry out last col; spill chunk
                nc.vector.tensor_copy(hcar[:, :, :], hnc[:, :, 512:513])
                nc.sync.dma_start(
                    out=hnTd[:, :].rearrange("(ci p) t -> p ci t", p=P)[:, :, j * 512:(j + 1) * 512],
                    in_=hnc[:, :, 1:513])
                # mixes
                xk_ = pl.tile([P, CT, 512], BF16, tag="xk", bufs=1, name=f"xk{j}")
                xv_ = pl.tile([P, CT, 512], BF16, tag="xv", bufs=1, name=f"xv{j}")
                xr_ = pl.tile([P, CT, 512], BF16, tag="xr", bufs=1, name=f"xr{j}")
                for ci in range(CT):
                    d_ = pl.tile([P, 512], BF16, tag="mixd", bufs=2, name=f"d{j}_{ci}")
                    nc.vector.tensor_sub(d_, hnc[:, ci, 1:513], hnc[:, ci, 0:512])
                    nc.vector.scalar_tensor_tensor(xk_[:, ci, :], d_, cvc(S_MK, ci), hnc[:, ci, 0:512], AL.mult, AL.add)
                    nc.vector.scalar_tensor_tensor(xv_[:, ci, :], d_, cvc(S_MV, ci), hnc[:, ci, 0:512], AL.mult, AL.add)
                    nc.vector.scalar_tensor_tensor(xr_[:, ci, :], d_, cvc(S_MR, ci), hnc[:, ci, 0:512], AL.mult, AL.add)
                # k/v/r matmuls + WKV
                rw_ = pl.tile([P, CT, 512], BF16, tag="rw", name=f"rw{j}", bufs=1)
                for co in range(CT):
                    pk_ = pp.tile([P, 512], F32, tag="p0", name=f"pk{j}_{co}")
                    pv_ = pp.tile([P, 512], F32, tag="p1", name=f"pv{j}_{co}")
                    pr_ = pp.tile([P, 512], F32, tag="p2", name=f"pr{j}_{co}")
                    for ci in range(CT):
                        nc.tensor.matmul(pk_, wk_sb[:, ci, co * P:(co + 1) * P], xk_[:, ci, :],
                                         start=(ci == 0), stop=(ci == CT - 1))
                    for ci in range(CT):
                        nc.tensor.matmul(pv_, wv_sb[:, ci, co * P:(co + 1) * P], xv_[:, ci, :],
                                         start=(ci == 0), stop=(ci == CT - 1))
                    for ci in range(CT):
                        nc.tensor.matmul(pr_, wr_sb[:, ci, co * P:(co + 1) * P], xr_[:, ci, :],
                                         start=(ci == 0), stop=(ci == CT - 1))
                    if j == 0:
                        nc.vector.tensor_scalar_add(pk_[:, 0:1], pk_[:, 0:1], cvc(S_FK, co))
                        nc.vector.tensor_scalar_add(pv_[:, 0:1], pv_[:, 0:1], cvc(S_FV, co))
                        nc.vector.tensor_scalar_add(pr_[:, 0:1], pr_[:, 0:1], cvc(S_FR, co))
                    for h in range(2):
                        sl = slice(h * 256, (h + 1) * 256)
                        ek_ = pl.tile([P, 256], F32, tag="ek", bufs=2, name=f"ek{j}_{co}_{h}")
                        nc.scalar.activation(ek_, pk_[:, sl], ACT.Exp, bias=cvc(S_KB, co))
                        ekv_ = pl.tile([P, 256], F32, tag="ekv", bufs=2, name=f"ekv{j}_{co}_{h}")
                        nc.vector.scalar_tensor_tensor(ekv_, pv_[:, sl], cvc(S_VB, co), ek_, AL.add, AL.mult)
                        a_ = pl.tile([P, 257], F32, tag="a", bufs=2, name=f"a{j}_{co}_{h}")
                        b_ = pl.tile([P, 257], F32, tag="b", bufs=2, name=f"b{j}_{co}_{h}")
                        if j == 0 and h == 0:
                            nc.vector.memset(a_[:, 0:1], 0.0)
                            nc.vector.memset(b_[:, 0:1], 0.0)
                        else:
                            nc.vector.tensor_copy(a_[:, 0:1], carA[:, co:co + 1])
                            nc.vector.tensor_copy(b_[:, 0:1], carB[:, co:co + 1])
                        lam_bc = _bcast_free(cvc(S_LAM, co), 256)
                        nc.vector.tensor_tensor_scan(a_[:, 1:257], lam_bc, ekv_, a_[:, 0:1], AL.mult, AL.add)
                        nc.vector.tensor_tensor_scan(b_[:, 1:257], lam_bc, ek_, b_[:, 0:1], AL.mult, AL.add)
                        nc.vector.tensor_copy(carA[:, co:co + 1], a_[:, 256:257])
                        nc.vector.tensor_copy(carB[:, co:co + 1], b_[:, 256:257])
                        # num (in-place on ekv_), den (in-place on ek_)
                        nc.vector.scalar_tensor_tensor(ekv_, ekv_, cvc(S_EU, co), a_[:, 0:256], AL.mult, AL.add)
                        nc.vector.scalar_tensor_tensor(ek_, ek_, cvc(S_EU, co), b_[:, 0:256], AL.mult, AL.add)
                        nc.vector.reciprocal_approx_fast(out=ek_, in_=ek_)
                        nc.vector.tensor_mul(ekv_, ekv_, ek_)
                        srt_ = pl.tile([P, 256], F32, tag="srt", bufs=2, name=f"srt{j}_{co}_{h}")
                        nc.scalar.activation(srt_, pr_[:, sl], ACT.Sigmoid, bias=cvc(S_RB, co))
                        nc.vector.tensor_mul(rw_[:, co, sl], ekv_, srt_)
                nc.sync.dma_start(
                    out=rwTd[:, :].rearrange("(ci p) t -> p ci t", p=P)[:, :, j * 512:(j + 1) * 512],
                    in_=rw_)

            # ---- weights: phase 2 (reuse w2m slots) ----
            wo_sb = pl.tile([P, CT, C], BF16, tag="w2m", bufs=3)
            nc.sync.dma_start(out=wo_sb, in_=woT_d[:, :].rearrange("(ci p) co -> p ci co", p=P))
            sh_sb = pl.tile([P, CT, C], BF16, tag="w2m", bufs=3)
            nc.sync.dma_start(out=sh_sb, in_=shT_d[:, :].rearrange("(ci p) co -> p ci co", p=P))
            fwr_sb = pl.tile([P, CT, C], BF16, tag="w2m", bufs=3)
            nc.sync.dma_start(out=fwr_sb, in_=fwrT_d[:, :].rearrange("(ci p) co -> p ci co", p=P))
            fwk_sb = pl.tile([P, CT, F], BF16, tag="w8")
            nc.sync.dma_start(out=fwk_sb, in_=fwkT_d[:, :].rearrange("(ci p) fo -> p ci fo", p=P))

            # ================= Phase ATT + FFN-A =================
            for j in range(NJ):
                rwin = pl.tile([P, CT, 512], BF16, tag="rw", name=f"rwi{j}", bufs=1)
                nc.sync.dma_start(
                    in_=rwTd[:, :].rearrange("(ci p) t -> p ci t", p=P)[:, :, j * 512:(j + 1) * 512],
                    out=rwin)
                hnin = pl.tile([P, CT, 512], BF16, tag="hnT", bufs=2, name=f"hni{j}")
                nc.sync.dma_start(
                    in_=hnTd[:, :].rearrange("(ci p) t -> p ci t", p=P)[:, :, j * 512:(j + 1) * 512],
                    out=hnin)
                gnc = pl.tile([P, CT, 513], BF16, tag="hnT", bufs=2, name=f"gnc{j}")
                if j == 0:
                    nc.vector.memset(gnc[:, :, 0:1], 0.0)
                else:
                    nc.vector.tensor_copy(gnc[:, :, 0:1], gcar[:, :, :])
                for tl in range(4):
                    tt = 4 * j + tl
                    o1 = pl.tile([P, C], F32, tag="xin", bufs=2, name=f"o1_{tt}")
                    for nco in range(NC2):
                        ps_ = pp.tile([P, 512], F32, tag="p0", name=f"ps{tt}_{nco}")
                        pw_ = pp.tile([P, 512], F32, tag="p1", name=f"pw{tt}_{nco}")
                        for ci in range(CT):
                            nc.tensor.matmul(ps_, hnin[:, ci, tl * P:(tl + 1) * P],
                                             sh_sb[:, ci, nco * 512:(nco + 1) * 512],
                                             start=(ci == 0), stop=(ci == CT - 1))
                        for ci in range(CT):
                            nc.tensor.matmul(pw_, rwin[:, ci, tl * P:(tl + 1) * P],
                                             wo_sb[:, ci, nco * 512:(nco + 1) * 512],
                                             start=(ci == 0), stop=(ci == CT - 1))
                        tmp1 = pl.tile([P, 512], F32, tag="at1", bufs=2, name=f"at{tt}_{nco}")
                        nc.scalar.mul(tmp1, ps_, musd[:, 1, tt:tt + 1])
                        nc.vector.scalar_tensor_tensor(pw_, srow_bc[:, nco * 512:(nco + 1) * 512],
                                                       musd[:, 0, tt:tt + 1], pw_, AL.mult, AL.add)
                        nc.vector.tensor_add(o1[:, nco * 512:(nco + 1) * 512], tmp1, pw_)
                    nc.sync.dma_start(out=o1d[tt * P:(tt + 1) * P, :], in_=o1)
                    # ln2
                    st2 = pl.tile([P, 2, 6], F32, tag="st", bufs=2, name=f"st2_{tt}")
                    nc.vector.bn_stats(out=st2[:, 0, :], in_=o1[:, 0:512])
                    nc.vector.bn_stats(out=st2[:, 1, :], in_=o1[:, 512:1024])
                    mv2 = pl.tile([P, 2], F32, tag="mv", bufs=2, name=f"mv2_{tt}")
                    nc.vector.bn_aggr(out=mv2, in_=st2)
                    sd2 = pl.tile([P, 1], F32, tag="sd2", bufs=2, name=f"sd2_{tt}")
                    nc.scalar.activation(sd2, mv2[:, 1:2], ACT.Sqrt, bias=epst)
                    rstd2 = pl.tile([P, 1], F32, tag="rstd", bufs=2, name=f"rstd2_{tt}")
                    nc.vector.reciprocal(rstd2, sd2)
                    gnb = pl.tile([P, C], BF16, tag="hnn", bufs=2, name=f"gnb{tt}")
                    nc.vector.tensor_scalar(gnb, o1, mv2[:, 0:1], rstd2, AL.subtract, AL.mult)
                    for ci in range(CT):
                        trs2 = pl.tile([P, P], BF16, tag="trs", bufs=3, name=f"trs2_{tt}_{ci}")
                        nc.sync.dma_start(out=trs2, in_=gnb[:, ci * P:(ci + 1) * P], transpose=True)
                        nc.gpsimd.tensor_copy(gnc[:, ci, 1 + tl * P:1 + (tl + 1) * P], trs2)
                nc.vector.tensor_copy(gcar[:, :, :], gnc[:, :, 512:513])
                # ffn mixes
                gk_ = pl.tile([P, CT, 512], BF16, tag="xk", name=f"gk{j}", bufs=1)
                gr_ = pl.tile([P, CT, 512], BF16, tag="xv", bufs=1, name=f"gr{j}")
                for ci in range(CT):
                    d2 = pl.tile([P, 512], BF16, tag="mixd", bufs=2, name=f"d2_{j}_{ci}")
                    nc.vector.tensor_sub(d2, gnc[:, ci, 1:513], gnc[:, ci, 0:512])
                    nc.vector.scalar_tensor_tensor(gk_[:, ci, :], d2, cvc(S_FFK, ci), gnc[:, ci, 0:512], AL.mult, AL.add)
                    nc.vector.scalar_tensor_tensor(gr_[:, ci, :], d2, cvc(S_FFR, ci), gnc[:, ci, 0:512], AL.mult, AL.add)
                # kk = relu(fwk @ gk)^2 -> spill
                for ft in range(FT):
                    pkk = pp.tile([P, 512], F32, tag="p2", name=f"pkk{j}_{ft}")
                    for ci in range(CT):
                        nc.tensor.matmul(pkk, fwk_sb[:, ci, ft * P:(ft + 1) * P], gk_[:, ci, :],
                                         start=(ci == 0), stop=(ci == CT - 1))
                    if j == 0:
                        nc.vector.tensor_scalar_add(pkk[:, 0:1], pkk[:, 0:1], fv[:, 1, ft:ft + 1])
                    kr = pl.tile([P, 512], F32, tag="kr", bufs=2, name=f"kr{j}_{ft}")
                    nc.scalar.activation(kr, pkk, ACT.Relu, bias=fv[:, 0, ft:ft + 1])
                    k2 = pl.tile([P, 512], BF16, tag="k2", bufs=2, name=f"k2_{j}_{ft}")
                    nc.vector.tensor_mul(k2, kr, kr)
                    nc.sync.dma_start(out=kk2d[ft * P:(ft + 1) * P, j * 512:(j + 1) * 512], in_=k2)
                # rr -> sigmoid -> srT spill
                for co in range(CT):
                    prr = pp.tile([P, 512], F32, tag="p3", name=f"prr{j}_{co}")
                    for ci in range(CT):
                        nc.tensor.matmul(prr, fwr_sb[:, ci, co * P:(co + 1) * P], gr_[:, ci, :],
                                         start=(ci == 0), stop=(ci == CT - 1))
                    if j == 0:
                        nc.vector.tensor_scalar_add(prr[:, 0:1], prr[:, 0:1], cvc(S_FRR, co))
                    srtc = pl.tile([P, 512], BF16, tag="sc", bufs=2, name=f"sc{j}_{co}")
                    nc.scalar.activation(srtc, prr, ACT.Sigmoid, bias=cvc(S_RRB, co))
                    nc.sync.dma_start(out=srTd[co * P:(co + 1) * P, j * 512:(j + 1) * 512], in_=srtc)

            # ---- weights: phase 3 ----
            fwv_sb = pl.tile([P, FT, C], BF16, tag="w8")
            nc.sync.dma_start(out=fwv_sb, in_=fwvT_d[:, :].rearrange("(fi p) co -> p fi co", p=P))

            # ================= Phase KV (final) =================
            for tt in range(TT):
                kc0 = pl.tile([P, 16, P], BF16, tag="xk", name=f"kc0_{tt}", bufs=1)
                nc.sync.dma_start(out=kc0, in_=kk2d[0:2048, tt * P:(tt + 1) * P].rearrange("(f p) t -> p f t", p=P))
                kc1 = pl.tile([P, 16, P], BF16, tag="xv", bufs=1, name=f"kc1_{tt}")
                nc.sync.dma_start(out=kc1, in_=kk2d[2048:4096, tt * P:(tt + 1) * P].rearrange("(f p) t -> p f t", p=P))
                o1r = pl.tile([P, C], F32, tag="xin", bufs=2, name=f"o1r{tt}")
                nc.sync.dma_start(out=o1r, in_=o1d[tt * P:(tt + 1) * P, :])
                srn = pl.tile([P, C], BF16, tag="srn", bufs=2, name=f"srn{tt}")
                for co in range(CT):
                    trs3 = pl.tile([P, P], BF16, tag="trs", bufs=3, name=f"trs3_{tt}_{co}")
                    nc.sync.dma_start(out=trs3,
                                        in_=srTd[co * P:(co + 1) * P, tt * P:(tt + 1) * P],
                                        transpose=True)
                    nc.gpsimd.tensor_copy(srn[:, co * P:(co + 1) * P], trs3)
                for nco in range(NC2):
                    pkv = pp.tile([P, 512], F32, tag="p0", name=f"pkv{tt}_{nco}")
                    for ft in range(FT):
                        lhs = kc0 if ft < 16 else kc1
                        nc.tensor.matmul(pkv, lhs[:, ft % 16, :], fwv_sb[:, ft, nco * 512:(nco + 1) * 512],
                                         start=(ft == 0), stop=(ft == FT - 1))
                    tmpv = pl.tile([P, 512], F32, tag="kvt", bufs=1, name=f"kvt{tt}_{nco}")
                    nc.vector.tensor_mul(tmpv, pkv, srn[:, nco * 512:(nco + 1) * 512])
                    nc.vector.tensor_add(o1r[:, nco * 512:(nco + 1) * 512],
                                         o1r[:, nco * 512:(nco + 1) * 512], tmpv)
                nc.sync.dma_start(out=out_d[tt * P:(tt + 1) * P, :], in_=o1r)

    nc.compile()
    return nc


_NC_CACHE = {}


def get_nc(T):
    if T not in _NC_CACHE:
        _NC_CACHE[T] = build_nc(T)
    return _NC_CACHE[T]


def host_prep(inp, T):
    """Build per-core in_maps from full inputs (float64 math on host)."""
    f8 = lambda a: np.asarray(a, np.float64)
    x = np.asarray(inp["x"], np.float32)
    w1, b1 = f8(inp["ln1_w"]), f8(inp["ln1_b"])
    w2, b2 = f8(inp["ln2_w"]), f8(inp["ln2_b"])
    Wk, Wv, Wr, Wo = f8(inp["att_Wk"]), f8(inp["att_Wv"]), f8(inp["att_Wr"]), f8(inp["att_Wo"])
    Wsh = f8(inp["short_W"])
    fWk, fWr, fWv = f8(inp["ffn_Wk"]), f8(inp["ffn_Wr"]), f8(inp["ffn_Wv"])
    mk, mvx, mr = f8(inp["att_mix_k"]), f8(inp["att_mix_v"]), f8(inp["att_mix_r"])
    fk, fr = f8(inp["ffn_mix_k"]), f8(inp["ffn_mix_r"])
    decay, first = f8(inp["att_time_decay"]), f8(inp["att_time_first"])

    def pack_c(v):
        return np.asarray(v, np.float32).reshape(CT, P).T  # [128, CT]

    lam = np.exp(-np.exp(decay))
    eu = np.exp(first)
    kbias = Wk @ b1
    vbias = Wv @ b1
    rbias = Wr @ b1
    fixk = -Wk @ ((1.0 - mk) * b1)
    fixv = -Wv @ ((1.0 - mvx) * b1)
    fixr = -Wr @ ((1.0 - mr) * b1)
    kkbias = fWk @ b2
    fixkk = -fWk @ ((1.0 - fk) * b2)
    rrbias = fWr @ b2
    fixrr = -fWr @ ((1.0 - fr) * b2)
    srow = Wsh.sum(axis=1)

    cvec = np.stack([pack_c(v) for v in
                     [lam, eu, mk, mvx, mr, kbias, vbias, rbias,
                      fixk, fixv, fixr, fk, fr, rrbias, fixrr]], axis=1)  # [128, 15, 8]
    fvec = np.stack([np.asarray(v, np.float32).reshape(FT, P).T for v in [kkbias, fixkk]],
                    axis=1)  # [128, 2, 32]

    shared = {
        "wkT": np.ascontiguousarray((Wk * w1[None, :]).T.astype(BF)),
        "wvT": np.ascontiguousarray((Wv * w1[None, :]).T.astype(BF)),
        "wrT": np.ascontiguousarray((Wr * w1[None, :]).T.astype(BF)),
        "woT": np.ascontiguousarray(Wo.T.astype(BF)),
        "shT": np.ascontiguousarray(Wsh.T.astype(BF)),
        "fwkT": np.ascontiguousarray((fWk * w2[None, :]).T.astype(BF)),
        "fwrT": np.ascontiguousarray((fWr * w2[None, :]).T.astype(BF)),
        "fwvT": np.ascontiguousarray(fWv.T.astype(BF)),
        "cvec": np.ascontiguousarray(cvec.astype(np.float32)),
        "fvec": np.ascontiguousarray(fvec.astype(np.float32)),
        "srow": np.ascontiguousarray(srow.reshape(1, C).astype(BF)),
    }
    in_maps = []
    for b in range(x.shape[0]):
        m = dict(shared)
        m["x"] = np.ascontiguousarray(x[b, :T, :])
        in_maps.append(m)
    return in_maps


def kernel(**inputs):
    T = 2048
    nc = get_nc(T)
    in_maps = host_prep(inputs, T)
    res = run_bass_kernel_spmd(nc, in_maps, core_ids=list(range(len(in_maps))))
    out = np.stack([r["out"] for r in res.results], axis=0)
    return out.astype(np.float32)



# revision 13
# speedup vs baseline: 1.2803x; 1.2803x over previous
"""RWKV-style Block kernel for 8 Trainium2 NeuronCores (batch-parallel SPMD).

v2: interleaved software pipeline for TimeMix + ATT/LN2/FFN-front, then an
FFN-back phase. All transposes on the PE (identity matmul) instead of DMA
transposes; Act table switches batched (Ln/Exp for LN rstd, sigmoid batches);
hn/rw/gn kept chunk-local in SBUF; only gkT/grT/o1 spilled to DRAM.

Per core (one batch element b, T=2048, C=1024, F=4096), chunks of 512 tokens:
  A1(j):  ln1 stats -> hn bf16 [t,c] -> PE-transpose -> hnc[j] [c,t]
  B1(j-1): mixes xk/xv/xr -> r matmuls -> sigmoid sr -> k/v matmuls
           -> Exp/WKV linear recurrence (tensor_tensor_scan) -> rw [c,t]
  A2(j-1): wo/short matmuls [t,c] -> o1 (+ln1 affine fixup) -> spill o1
           -> ln2 -> gn -> PE-transpose -> mixes gk/gr -> spill gkT,grT
  phase 3 per chunk: fwr matmul -> sigmoid sr2 -> PE-transpose [t,c];
           kk2 = relu(fwk@gk + b)^2 (bf16) -> kv matmuls -> out = o1 + sr2*kv
"""
import numpy as np
import ml_dtypes

import concourse.bass as bass
import concourse.bacc as bacc
import concourse.mybir as mybir
import concourse.tile as tile
from concourse.bass_utils import run_bass_kernel_spmd

F32 = mybir.dt.float32
BF16 = mybir.dt.bfloat16
AL = mybir.AluOpType
ACT = mybir.ActivationFunctionType
BF = ml_dtypes.bfloat16

B, C, F = 8, 1024, 4096
P = 128
CT = C // P          # 8 c-tiles
FT = F // P          # 32 f-tiles
NC2 = C // 512       # 2
EPS = 1e-5

S_LAM, S_EU, S_MK, S_MV, S_MR, S_KB, S_VB, S_RB, S_FK, S_FV, S_FR, S_FFK, S_FFR, S_RRB, S_FRR = range(15)
NSLOT = 15


def _bcast_free(col_ap, n):
    """per-partition [128,1] column AP -> [128,n] stride-0 broadcast AP."""
    return bass.AP(tensor=col_ap.tensor, offset=col_ap.offset,
                   ap=[col_ap.ap[0], [0, n]])


def build_nc(T):
    NJ = T // 512
    TT = T // 128
    nc = bacc.Bacc("TRN2", target_bir_lowering=False)

    # ---------------- DRAM I/O ----------------
    x_d = nc.dram_tensor("x", [T, C], F32, kind="ExternalInput")
    wkT_d = nc.dram_tensor("wkT", [C, C], BF16, kind="ExternalInput")
    wvT_d = nc.dram_tensor("wvT", [C, C], BF16, kind="ExternalInput")
    wrT_d = nc.dram_tensor("wrT", [C, C], BF16, kind="ExternalInput")
    woT_d = nc.dram_tensor("woT", [C, C], BF16, kind="ExternalInput")
    shT_d = nc.dram_tensor("shT", [C, C], BF16, kind="ExternalInput")
    fwkT_d = nc.dram_tensor("fwkT", [C, F], BF16, kind="ExternalInput")
    fwrT_d = nc.dram_tensor("fwrT", [C, C], BF16, kind="ExternalInput")
    fwvT_d = nc.dram_tensor("fwvT", [F, C], BF16, kind="ExternalInput")
    cvec_d = nc.dram_tensor("cvec", [P, NSLOT, CT], F32, kind="ExternalInput")
    fvec_d = nc.dram_tensor("fvec", [P, 2, FT], F32, kind="ExternalInput")
    srow_d = nc.dram_tensor("srow", [1, C], BF16, kind="ExternalInput")
    ident_d = nc.dram_tensor("ident", [P, P], BF16, kind="ExternalInput")
    out_d = nc.dram_tensor("out", [T, C], F32, kind="ExternalOutput")

    # DRAM scratch (spills for phase 3)
    o1d = nc.dram_tensor("o1d", [T, C], F32)
    gkTd = nc.dram_tensor("gkTd", [C, T], BF16)
    grTd = nc.dram_tensor("grTd", [C, T], BF16)

    with tile.TileContext(nc) as tc:
        with tc.tile_pool(name="const", bufs=1) as pc, \
             tc.tile_pool(name="psum", bufs=2, space="PSUM") as pp:

            # ---- constants / carries (outer pool, live whole kernel) ----
            cv = pc.tile([P, NSLOT, CT], F32, tag="cv")
            nc.sync.dma_start(out=cv, in_=cvec_d[:, :, :])
            fv = pc.tile([P, 2, FT], F32, tag="fv")
            nc.sync.dma_start(out=fv, in_=fvec_d[:, :, :])
            srow_bc = pc.tile([P, C], BF16, tag="srow")
            s_ap = srow_d[0:1, :]
            nc.sync.dma_start(out=srow_bc, in_=bass.AP(
                tensor=s_ap.tensor, offset=s_ap.offset, ap=[[0, P], s_ap.ap[1]]))
            ident = pc.tile([P, P], BF16, tag="ident")
            nc.sync.dma_start(out=ident, in_=ident_d[:, :])
            epst = pc.tile([P, 1], F32, tag="epst")
            nc.vector.memset(epst, EPS)
            carA = pc.tile([P, CT], F32, tag="carA")
            carB = pc.tile([P, CT], F32, tag="carB")
            mus = pc.tile([P, 2, TT], F32, tag="mus")
            gcar = pc.tile([P, CT, 1], BF16, tag="gcar")

            def cvc(slot, ci):
                return cv[:, slot, ci:ci + 1]

            # =========================================================
            # main interleaved pipeline (pool_main)
            # =========================================================
            with tc.tile_pool(name="main", bufs=1) as pl:
                wk_sb = pl.tile([P, CT, C], BF16, tag="wk")
                nc.sync.dma_start(out=wk_sb, in_=wkT_d[:, :].rearrange("(ci p) co -> p ci co", p=P))
                wv_sb = pl.tile([P, CT, C], BF16, tag="wv")
                nc.sync.dma_start(out=wv_sb, in_=wvT_d[:, :].rearrange("(ci p) co -> p ci co", p=P))
                wr_sb = pl.tile([P, CT, C], BF16, tag="wr")
                nc.sync.dma_start(out=wr_sb, in_=wrT_d[:, :].rearrange("(ci p) co -> p ci co", p=P))
                wo_sb = pl.tile([P, CT, C], BF16, tag="wo")
                nc.sync.dma_start(out=wo_sb, in_=woT_d[:, :].rearrange("(ci p) co -> p ci co", p=P))
                sh_sb = pl.tile([P, CT, C], BF16, tag="sh")
                nc.sync.dma_start(out=sh_sb, in_=shT_d[:, :].rearrange("(ci p) co -> p ci co", p=P))

                hn_t = {}

                def stage_A1(j):
                    hnc = pl.tile([P, CT, 513], BF16, tag="hnc", bufs=2, name=f"hnc{j}")
                    hn_t[j] = hnc
                    if j == 0:
                        nc.vector.memset(hnc[:, :, 0:1], 0.0)
                    else:
                        nc.gpsimd.tensor_copy(hnc[:, :, 0:1], hn_t[j - 1][:, :, 512:513])
                    for tl in range(4):
                        tt = 4 * j + tl
                        xt = pl.tile([P, C], F32, tag="xin", bufs=2, name=f"xt{tt}")
                        nc.sync.dma_start(out=xt, in_=x_d[tt * P:(tt + 1) * P, :])
                        st = pl.tile([P, 2, 6], F32, tag="st", bufs=2, name=f"st{tt}")
                        nc.vector.bn_stats(out=st[:, 0, :], in_=xt[:, 0:512])
                        nc.vector.bn_stats(out=st[:, 1, :], in_=xt[:, 512:1024])
                        mv = pl.tile([P, 2], F32, tag="mv", bufs=2, name=f"mv{tt}")
                        nc.vector.bn_aggr(out=mv, in_=st)
                        nc.vector.tensor_copy(mus[:, 0, tt:tt + 1], mv[:, 0:1])
                        lv = pl.tile([P, 1], F32, tag="lv", bufs=2, name=f"lv{tt}")
                        nc.scalar.activation(lv, mv[:, 1:2], ACT.Ln, bias=epst)
                        rstd = pl.tile([P, 1], F32, tag="rstd", bufs=2, name=f"rstd{tt}")
                        nc.scalar.activation(rstd, lv, ACT.Exp, scale=-0.5)
                        nc.scalar.activation(mus[:, 1, tt:tt + 1], lv, ACT.Exp, scale=0.5)
                        negb = pl.tile([P, 1], F32, tag="negb", bufs=2, name=f"negb{tt}")
                        nc.vector.tensor_scalar(negb, mv[:, 0:1], rstd, -1.0, AL.mult, AL.mult)
                        hnb = pl.tile([P, C], BF16, tag="hnn", bufs=2, name=f"hnb{tt}")
                        nc.scalar.activation(hnb, xt, ACT.Identity, bias=negb, scale=rstd)
                        ptr = pp.tile([P, CT, P], BF16, tag="pt", name=f"ptr{tt}")
                        for ci in range(CT):
                            nc.tensor.transpose(ptr[:, ci, :], hnb[:, ci * P:(ci + 1) * P], ident)
                        nc.scalar.activation(hnc[:, :, 1 + tl * P:1 + (tl + 1) * P], ptr, ACT.Copy)

                def stage_B1(j):
                    hnc = hn_t[j]
                    xk = pl.tile([P, CT, 512], BF16, tag="xk", bufs=1, name=f"xk{j}")
                    xv = pl.tile([P, CT, 512], BF16, tag="xv", bufs=1, name=f"xv{j}")
                    xr = pl.tile([P, CT, 512], BF16, tag="xr", bufs=1, name=f"xr{j}")
                    for ci in range(CT):
                        d = pl.tile([P, 512], BF16, tag="mixd", bufs=2, name=f"d{j}_{ci}")
                        nc.gpsimd.tensor_sub(d, hnc[:, ci, 1:513], hnc[:, ci, 0:512])
                        nc.vector.scalar_tensor_tensor(xk[:, ci, :], d, cvc(S_MK, ci), hnc[:, ci, 0:512], AL.mult, AL.add)
                        nc.vector.scalar_tensor_tensor(xv[:, ci, :], d, cvc(S_MV, ci), hnc[:, ci, 0:512], AL.mult, AL.add)
                        nc.vector.scalar_tensor_tensor(xr[:, ci, :], d, cvc(S_MR, ci), hnc[:, ci, 0:512], AL.mult, AL.add)
                    # r matmuls + sigmoid (one Act sigmoid batch)
                    sra = pl.tile([P, CT, 512], BF16, tag="sr", bufs=1, name=f"sra{j}")
                    for co in range(CT):
                        pr = pp.tile([P, 512], F32, tag="p2", name=f"pr{j}_{co}")
                        for ci in range(CT):
                            nc.tensor.matmul(pr, wr_sb[:, ci, co * P:(co + 1) * P], xr[:, ci, :],
                                             start=(ci == 0), stop=(ci == CT - 1))
                        if j == 0:
                            nc.vector.tensor_scalar_add(pr[:, 0:1], pr[:, 0:1], cvc(S_FR, co))
                        nc.scalar.activation(sra[:, co, :], pr, ACT.Sigmoid, bias=cvc(S_RB, co))
                    # k/v matmuls + Exp batch + WKV
                    rw = pl.tile([P, CT, 512], BF16, tag="rw", bufs=1, name=f"rw{j}")
                    for co in range(CT):
                        pk = pp.tile([P, 512], F32, tag="p0", name=f"pk{j}_{co}")
                        pv = pp.tile([P, 512], F32, tag="p1", name=f"pv{j}_{co}")
                        for ci in range(CT):
                            nc.tensor.matmul(pk, wk_sb[:, ci, co * P:(co + 1) * P], xk[:, ci, :],
                                             start=(ci == 0), stop=(ci == CT - 1))
                        for ci in range(CT):
                            nc.tensor.matmul(pv, wv_sb[:, ci, co * P:(co + 1) * P], xv[:, ci, :],
                                             start=(ci == 0), stop=(ci == CT - 1))
                        if j == 0:
                            nc.vector.tensor_scalar_add(pk[:, 0:1], pk[:, 0:1], cvc(S_FK, co))
                            nc.vector.tensor_scalar_add(pv[:, 0:1], pv[:, 0:1], cvc(S_FV, co))
                        ek = pl.tile([P, 512], F32, tag="ek", bufs=2, name=f"ek{j}_{co}")
                        nc.scalar.activation(ek, pk, ACT.Exp, bias=cvc(S_KB, co))
                        ekv = pl.tile([P, 512], F32, tag="ekv", bufs=2, name=f"ekv{j}_{co}")
                        nc.vector.scalar_tensor_tensor(ekv, pv, cvc(S_VB, co), ek, AL.add, AL.mult)
                        a_ = pl.tile([P, 513], F32, tag="a", bufs=2, name=f"a{j}_{co}")
                        b_ = pl.tile([P, 513], F32, tag="b", bufs=2, name=f"b{j}_{co}")
                        if j == 0:
                            nc.vector.memset(a_[:, 0:1], 0.0)
                            nc.vector.memset(b_[:, 0:1], 0.0)
                        else:
                            nc.vector.tensor_copy(a_[:, 0:1], carA[:, co:co + 1])
                            nc.vector.tensor_copy(b_[:, 0:1], carB[:, co:co + 1])
                        lam_bc = _bcast_free(cvc(S_LAM, co), 512)
                        nc.vector.tensor_tensor_scan(a_[:, 1:513], lam_bc, ekv, a_[:, 0:1], AL.mult, AL.add)
                        nc.vector.tensor_tensor_scan(b_[:, 1:513], lam_bc, ek, b_[:, 0:1], AL.mult, AL.add)
                        nc.vector.tensor_copy(carA[:, co:co + 1], a_[:, 512:513])
                        nc.vector.tensor_copy(carB[:, co:co + 1], b_[:, 512:513])
                        # num (in-place ekv) on DVE, den (in-place ek) on Pool
                        nc.vector.scalar_tensor_tensor(ekv, ekv, cvc(S_EU, co), a_[:, 0:512], AL.mult, AL.add)
                        nc.gpsimd.tensor_mul(ek, ek, _bcast_free(cvc(S_EU, co), 512))
                        nc.gpsimd.tensor_add(ek, ek, b_[:, 0:512])
                        nc.vector.reciprocal(ek, ek)
                        nc.gpsimd.tensor_mul(ekv, ekv, ek)
                        nc.gpsimd.tensor_mul(rw[:, co, :], ekv, sra[:, co, :])
                    return rw

                def stage_A2(j, rw):
                    hnc = hn_t[j]
                    gnc = pl.tile([P, CT, 513], BF16, tag="gnc", bufs=1, name=f"gnc{j}")
                    if j == 0:
                        nc.vector.memset(gnc[:, :, 0:1], 0.0)
                    else:
                        nc.gpsimd.tensor_copy(gnc[:, :, 0:1], gcar[:, :, :])
                    for tl in range(4):
                        tt = 4 * j + tl
                        o1 = pl.tile([P, C], F32, tag="o1", bufs=2, name=f"o1_{tt}")
                        for nco in range(NC2):
                            ps = pp.tile([P, 512], F32, tag="p0", name=f"ps{tt}_{nco}")
                            pw = pp.tile([P, 512], F32, tag="p1", name=f"pw{tt}_{nco}")
                            for ci in range(CT):
                                nc.tensor.matmul(ps, hnc[:, ci, 1 + tl * P:1 + (tl + 1) * P],
                                                 sh_sb[:, ci, nco * 512:(nco + 1) * 512],
                                                 start=(ci == 0), stop=(ci == CT - 1))
                            for ci in range(CT):
                                nc.tensor.matmul(pw, rw[:, ci, tl * P:(tl + 1) * P],
                                                 wo_sb[:, ci, nco * 512:(nco + 1) * 512],
                                                 start=(ci == 0), stop=(ci == CT - 1))
                            o1s = o1[:, nco * 512:(nco + 1) * 512]
                            nc.scalar.activation(o1s, ps, ACT.Identity, scale=mus[:, 1, tt:tt + 1])
                            nc.vector.tensor_add(o1s, o1s, pw)
                            nc.vector.scalar_tensor_tensor(o1s, srow_bc[:, nco * 512:(nco + 1) * 512],
                                                           mus[:, 0, tt:tt + 1], o1s, AL.mult, AL.add)
                        nc.sync.dma_start(out=o1d[tt * P:(tt + 1) * P, :], in_=o1)
                        st2 = pl.tile([P, 2, 6], F32, tag="st", bufs=2, name=f"st2_{tt}")
                        nc.vector.bn_stats(out=st2[:, 0, :], in_=o1[:, 0:512])
                        nc.vector.bn_stats(out=st2[:, 1, :], in_=o1[:, 512:1024])
                        mv2 = pl.tile([P, 2], F32, tag="mv", bufs=2, name=f"mv2_{tt}")
                        nc.vector.bn_aggr(out=mv2, in_=st2)
                        lv2 = pl.tile([P, 1], F32, tag="lv", bufs=2, name=f"lv2_{tt}")
                        nc.scalar.activation(lv2, mv2[:, 1:2], ACT.Ln, bias=epst)
                        rstd2 = pl.tile([P, 1], F32, tag="rstd", bufs=2, name=f"rstd2_{tt}")
                        nc.scalar.activation(rstd2, lv2, ACT.Exp, scale=-0.5)
                        negb2 = pl.tile([P, 1], F32, tag="negb", bufs=2, name=f"negb2_{tt}")
                        nc.vector.tensor_scalar(negb2, mv2[:, 0:1], rstd2, -1.0, AL.mult, AL.mult)
                        gnb = pl.tile([P, C], BF16, tag="hnn", bufs=2, name=f"gnb{tt}")
                        nc.scalar.activation(gnb, o1, ACT.Identity, bias=negb2, scale=rstd2)
                        ptr2 = pp.tile([P, CT, P], BF16, tag="pt", name=f"ptr2_{tt}")
                        for ci in range(CT):
                            nc.tensor.transpose(ptr2[:, ci, :], gnb[:, ci * P:(ci + 1) * P], ident)
                        nc.scalar.activation(gnc[:, :, 1 + tl * P:1 + (tl + 1) * P], ptr2, ACT.Copy)
                    nc.gpsimd.tensor_copy(gcar[:, :, :], gnc[:, :, 512:513])
                    gk = pl.tile([P, CT, 512], BF16, tag="gk", bufs=1, name=f"gk{j}")
                    gr = pl.tile([P, CT, 512], BF16, tag="gr", bufs=1, name=f"gr{j}")
                    for ci in range(CT):
                        d2 = pl.tile([P, 512], BF16, tag="mixd", bufs=2, name=f"d2_{j}_{ci}")
                        nc.gpsimd.tensor_sub(d2, gnc[:, ci, 1:513], gnc[:, ci, 0:512])
                        nc.vector.scalar_tensor_tensor(gk[:, ci, :], d2, cvc(S_FFK, ci), gnc[:, ci, 0:512], AL.mult, AL.add)
                        nc.vector.scalar_tensor_tensor(gr[:, ci, :], d2, cvc(S_FFR, ci), gnc[:, ci, 0:512], AL.mult, AL.add)
                    nc.sync.dma_start(
                        out=gkTd[:, :].rearrange("(ci p) t -> p ci t", p=P)[:, :, j * 512:(j + 1) * 512],
                        in_=gk)
                    nc.sync.dma_start(
                        out=grTd[:, :].rearrange("(ci p) t -> p ci t", p=P)[:, :, j * 512:(j + 1) * 512],
                        in_=gr)

                for j in range(NJ + 1):
                    if j < NJ:
                        stage_A1(j)
                    if j >= 1:
                        rw = stage_B1(j - 1)
                        stage_A2(j - 1, rw)

            # =========================================================
            # phase 3: FFN back (pool_p3)
            # =========================================================
            with tc.tile_pool(name="p3", bufs=1) as p3:
                fwr_sb = p3.tile([P, CT, C], BF16, tag="fwr")
                nc.sync.dma_start(out=fwr_sb, in_=fwrT_d[:, :].rearrange("(ci p) co -> p ci co", p=P))
                fwk_sb = p3.tile([P, CT, F], BF16, tag="fwk")
                nc.sync.dma_start(out=fwk_sb, in_=fwkT_d[:, :].rearrange("(ci p) fo -> p ci fo", p=P))
                fwv_sb = p3.tile([P, FT, C], BF16, tag="fwv")
                nc.sync.dma_start(out=fwv_sb, in_=fwvT_d[:, :].rearrange("(fi p) co -> p fi co", p=P))

                for j in range(NJ):
                    gki = p3.tile([P, CT, 512], BF16, tag="gki", bufs=1, name=f"gki{j}")
                    nc.sync.dma_start(
                        in_=gkTd[:, :].rearrange("(ci p) t -> p ci t", p=P)[:, :, j * 512:(j + 1) * 512],
                        out=gki)
                    gri = p3.tile([P, CT, 512], BF16, tag="gri", bufs=1, name=f"gri{j}")
                    nc.sync.dma_start(
                        in_=grTd[:, :].rearrange("(ci p) t -> p ci t", p=P)[:, :, j * 512:(j + 1) * 512],
                        out=gri)
                    # fwr matmul + sigmoid + transpose to [t,c]
                    s2t = p3.tile([P, 4, C], BF16, tag="s2t", bufs=1, name=f"s2t{j}")
                    for co in range(CT):
                        prr = pp.tile([P, 512], F32, tag="p2", name=f"prr{j}_{co}")
                        for ci in range(CT):
                            nc.tensor.matmul(prr, fwr_sb[:, ci, co * P:(co + 1) * P], gri[:, ci, :],
                                             start=(ci == 0), stop=(ci == CT - 1))
                        if j == 0:
                            nc.vector.tensor_scalar_add(prr[:, 0:1], prr[:, 0:1], cvc(S_FRR, co))
                        sr2 = p3.tile([P, 512], BF16, tag="sr2", bufs=2, name=f"sr2_{j}_{co}")
                        nc.scalar.activation(sr2, prr, ACT.Sigmoid, bias=cvc(S_RRB, co))
                        ptr3 = pp.tile([P, 4, P], BF16, tag="pt", name=f"ptr3_{j}_{co}")
                        for tl in range(4):
                            nc.tensor.transpose(ptr3[:, tl, :], sr2[:, tl * P:(tl + 1) * P], ident)
                        nc.scalar.activation(s2t[:, :, co * P:(co + 1) * P], ptr3, ACT.Copy)
                    # kk2 + kv, in two t-halves of 256
                    for h in range(2):
                        kk2 = p3.tile([P, FT, 256], BF16, tag="kk2", bufs=1, name=f"kk2_{j}_{h}")
                        hs = slice(h * 256, (h + 1) * 256)
                        for ft in range(FT):
                            pkk = pp.tile([P, 256], F32, tag="p0", name=f"pkk{j}_{h}_{ft}")
                            for ci in range(CT):
                                nc.tensor.matmul(pkk, fwk_sb[:, ci, ft * P:(ft + 1) * P], gki[:, ci, hs],
                                                 start=(ci == 0), stop=(ci == CT - 1))
                            if j == 0 and h == 0:
                                nc.vector.tensor_scalar_add(pkk[:, 0:1], pkk[:, 0:1], fv[:, 1, ft:ft + 1])
                            kr = p3.tile([P, 256], BF16, tag="kr", bufs=2, name=f"kr{j}_{h}_{ft}")
                            nc.scalar.activation(kr, pkk, ACT.Relu, bias=fv[:, 0, ft:ft + 1])
                            nc.vector.tensor_mul(kk2[:, ft, :], kr, kr)
                        for tlh in range(2):
                            tl = h * 2 + tlh
                            tt = 4 * j + tl
                            o1r = p3.tile([P, C], F32, tag="o1r", bufs=2, name=f"o1r{tt}")
                            nc.sync.dma_start(out=o1r, in_=o1d[tt * P:(tt + 1) * P, :])
                            for nco in range(NC2):
                                pkv = pp.tile([P, 512], F32, tag="p1", name=f"pkv{tt}_{nco}")
                                for ft in range(FT):
                                    nc.tensor.matmul(pkv, kk2[:, ft, tlh * P:(tlh + 1) * P],
                                                     fwv_sb[:, ft, nco * 512:(nco + 1) * 512],
                                                     start=(ft == 0), stop=(ft == FT - 1))
                                tmpv = p3.tile([P, 512], F32, tag="kvt", bufs=2, name=f"kvt{tt}_{nco}")
                                nc.vector.tensor_mul(tmpv, pkv, s2t[:, tl, nco * 512:(nco + 1) * 512])
                                nc.vector.tensor_add(o1r[:, nco * 512:(nco + 1) * 512],
                                                     o1r[:, nco * 512:(nco + 1) * 512], tmpv)
                            nc.sync.dma_start(out=out_d[tt * P:(tt + 1) * P, :], in_=o1r)

    nc.compile()
    return nc


_NC_CACHE = {}


def get_nc(T):
    if T not in _NC_CACHE:
        _NC_CACHE[T] = build_nc(T)
    return _NC_CACHE[T]


def host_prep(inp, T):
    """Build per-core in_maps from full inputs (float64 math on host)."""
    f8 = lambda a: np.asarray(a, np.float64)
    x = np.asarray(inp["x"], np.float32)
    w1, b1 = f8(inp["ln1_w"]), f8(inp["ln1_b"])
    w2, b2 = f8(inp["ln2_w"]), f8(inp["ln2_b"])
    Wk, Wv, Wr, Wo = f8(inp["att_Wk"]), f8(inp["att_Wv"]), f8(inp["att_Wr"]), f8(inp["att_Wo"])
    Wsh = f8(inp["short_W"])
    fWk, fWr, fWv = f8(inp["ffn_Wk"]), f8(inp["ffn_Wr"]), f8(inp["ffn_Wv"])
    mk, mvx, mr = f8(inp["att_mix_k"]), f8(inp["att_mix_v"]), f8(inp["att_mix_r"])
    fk, fr = f8(inp["ffn_mix_k"]), f8(inp["ffn_mix_r"])
    decay, first = f8(inp["att_time_decay"]), f8(inp["att_time_first"])

    def pack_c(v):
        return np.asarray(v, np.float32).reshape(CT, P).T  # [128, CT]

    lam = np.exp(-np.exp(decay))
    eu = np.exp(first)
    kbias = Wk @ b1
    vbias = Wv @ b1
    rbias = Wr @ b1
    fixk = -Wk @ ((1.0 - mk) * b1)
    fixv = -Wv @ ((1.0 - mvx) * b1)
    fixr = -Wr @ ((1.0 - mr) * b1)
    kkbias = fWk @ b2
    fixkk = -fWk @ ((1.0 - fk) * b2)
    rrbias = fWr @ b2
    fixrr = -fWr @ ((1.0 - fr) * b2)
    srow = Wsh.sum(axis=1)

    cvec = np.stack([pack_c(v) for v in
                     [lam, eu, mk, mvx, mr, kbias, vbias, rbias,
                      fixk, fixv, fixr, fk, fr, rrbias, fixrr]], axis=1)  # [128, 15, 8]
    fvec = np.stack([np.asarray(v, np.float32).reshape(FT, P).T for v in [kkbias, fixkk]],
                    axis=1)  # [128, 2, 32]

    shared = {
        "wkT": np.ascontiguousarray((Wk * w1[None, :]).T.astype(BF)),
        "wvT": np.ascontiguousarray((Wv * w1[None, :]).T.astype(BF)),
        "wrT": np.ascontiguousarray((Wr * w1[None, :]).T.astype(BF)),
        "woT": np.ascontiguousarray(Wo.T.astype(BF)),
        "shT": np.ascontiguousarray(Wsh.T.astype(BF)),
        "fwkT": np.ascontiguousarray((fWk * w2[None, :]).T.astype(BF)),
        "fwrT": np.ascontiguousarray((fWr * w2[None, :]).T.astype(BF)),
        "fwvT": np.ascontiguousarray(fWv.T.astype(BF)),
        "cvec": np.ascontiguousarray(cvec.astype(np.float32)),
        "fvec": np.ascontiguousarray(fvec.astype(np.float32)),
        "srow": np.ascontiguousarray(srow.reshape(1, C).astype(BF)),
        "ident": np.ascontiguousarray(np.eye(P, dtype=np.float32).astype(BF)),
    }
    in_maps = []
    for b in range(x.shape[0]):
        m = dict(shared)
        m["x"] = np.ascontiguousarray(x[b, :T, :])
        in_maps.append(m)
    return in_maps


def kernel(**inputs):
    T = 2048
    nc = get_nc(T)
    in_maps = host_prep(inputs, T)
    res = run_bass_kernel_spmd(nc, in_maps, core_ids=list(range(len(in_maps))))
    out = np.stack([r["out"] for r in res.results], axis=0)
    return out.astype(np.float32)


# revision 23
# speedup vs baseline: 1.4010x; 1.0942x over previous
"""RWKV-style Block kernel for 8 Trainium2 NeuronCores (batch-parallel SPMD).

v3: software-pipelined TimeMix + ATT/LN2/FFN-front main loop, then an FFN-back
phase. PE transposes (identity matmul) instead of DMA transposes; Act table
switches batched; deferred-LN2 scheduling so each chunk's normalization tail
overlaps the next chunk's matmuls; staged SBUF pools so phase-3 weights load
while the last chunk's tail still runs.

Per core (one batch element b, T=2048, C=1024, F=4096), chunks of 512 tokens:
  A1(j):  ln1 stats -> hn bf16 [t,c] -> PE-transpose -> hnc[j] [c,t]
  MIX(j): time-mix xk/xv/xr (hoisted one iteration early)
  B1a(j): r matmuls -> sigmoid sr ; B1b(j): short matmuls -> o1=ps*std
  B1c(j): k/v matmuls -> Exp -> WKV linear recurrence (tensor_tensor_scan)
  A2a(j): wo matmuls -> o1 += att + mu*srow -> spill o1
  A2b(j): (one iter later) ln2 -> gn -> PE-transpose -> ffn mixes -> spills
  phase 3 per chunk: fwr matmul -> sigmoid sr2 -> PE-transpose [t,c];
  kk2 = relu(fwk@gk+b)^2 -> kv matmuls -> out = o1 + sr2*kv
"""
import numpy as np
import ml_dtypes

import concourse.bass as bass
import concourse.bacc as bacc
import concourse.mybir as mybir
import concourse.tile as tile
from concourse.bass_utils import run_bass_kernel_spmd

F32 = mybir.dt.float32
BF16 = mybir.dt.bfloat16
AL = mybir.AluOpType
ACT = mybir.ActivationFunctionType
BF = ml_dtypes.bfloat16

B, C, F = 8, 1024, 4096
P = 128
CT = C // P          # 8 c-tiles
FT = F // P          # 32 f-tiles
NC2 = C // 512       # 2
EPS = 1e-5

S_LAM, S_EU, S_MK, S_MV, S_MR, S_KB, S_VB, S_RB, S_FK, S_FV, S_FR, S_FFK, S_FFR, S_RRB, S_FRR = range(15)
NSLOT = 15


def _bcast_free(col_ap, n):
    """per-partition [128,1] column AP -> [128,n] stride-0 broadcast AP."""
    return bass.AP(tensor=col_ap.tensor, offset=col_ap.offset,
                   ap=[col_ap.ap[0], [0, n]])


def build_nc(T):
    NJ = T // 512
    TT = T // 128
    nc = bacc.Bacc("TRN2", target_bir_lowering=False)

    # ---------------- DRAM I/O ----------------
    x_d = nc.dram_tensor("x", [T, C], F32, kind="ExternalInput")
    wkT_d = nc.dram_tensor("wkT", [C, C], BF16, kind="ExternalInput")
    wvT_d = nc.dram_tensor("wvT", [C, C], BF16, kind="ExternalInput")
    wrT_d = nc.dram_tensor("wrT", [C, C], BF16, kind="ExternalInput")
    woT_d = nc.dram_tensor("woT", [C, C], BF16, kind="ExternalInput")
    shT_d = nc.dram_tensor("shT", [C, C], BF16, kind="ExternalInput")
    fwkT_d = nc.dram_tensor("fwkT", [C, F], BF16, kind="ExternalInput")
    fwrT_d = nc.dram_tensor("fwrT", [C, C], BF16, kind="ExternalInput")
    fwvT_d = nc.dram_tensor("fwvT", [F, C], BF16, kind="ExternalInput")
    cvec_d = nc.dram_tensor("cvec", [P, NSLOT, CT], F32, kind="ExternalInput")
    fvec_d = nc.dram_tensor("fvec", [P, 2, FT], F32, kind="ExternalInput")
    srow_d = nc.dram_tensor("srow", [1, C], BF16, kind="ExternalInput")
    ident_d = nc.dram_tensor("ident", [P, P], BF16, kind="ExternalInput")
    out_d = nc.dram_tensor("out", [T, C], F32, kind="ExternalOutput")

    # DRAM scratch (spills for phase 3)
    o1d = nc.dram_tensor("o1d", [T, C], F32)
    gkTd = nc.dram_tensor("gkTd", [C, T], BF16)
    grTd = nc.dram_tensor("grTd", [C, T], BF16)

    with tile.TileContext(nc) as tc:
        with tc.tile_pool(name="const", bufs=1) as pc, \
             tc.tile_pool(name="psum", bufs=2, space="PSUM") as pp:

            # ---- constants / carries (outer pool, live whole kernel) ----
            cv = pc.tile([P, NSLOT, CT], F32, tag="cv")
            nc.sync.dma_start(out=cv, in_=cvec_d[:, :, :])
            fv = pc.tile([P, 2, FT], F32, tag="fv")
            nc.sync.dma_start(out=fv, in_=fvec_d[:, :, :])
            srow_bc = pc.tile([P, C], BF16, tag="srow")
            s_ap = srow_d[0:1, :]
            nc.sync.dma_start(out=srow_bc, in_=bass.AP(
                tensor=s_ap.tensor, offset=s_ap.offset, ap=[[0, P], s_ap.ap[1]]))
            ident = pc.tile([P, P], BF16, tag="ident")
            nc.sync.dma_start(out=ident, in_=ident_d[:, :])
            epst = pc.tile([P, 1], F32, tag="epst")
            nc.vector.memset(epst, EPS)
            carA = pc.tile([P, CT], F32, tag="carA")
            carB = pc.tile([P, CT], F32, tag="carB")
            mus = pc.tile([P, 2, TT], F32, tag="mus")
            gcar = pc.tile([P, CT, 1], BF16, tag="gcar")

            def cvc(slot, ci):
                return cv[:, slot, ci:ci + 1]

            # tail pool: tiles used by the deferred A2b stages (must outlive
            # the big main pool so the last A2b can overlap phase-3 loads)
            pt_pool = tc.alloc_tile_pool(name="tail", bufs=1)
            pb_pool = tc.alloc_tile_pool(name="big", bufs=1)
            pl = pb_pool
            ptl = pt_pool

            def ldw(pool, tag, dram):
                w = pool.tile([P, CT, C], BF16, tag=tag)
                r = dram[:, :].rearrange("(ci p) co -> p ci co", p=P)
                nc.sync.dma_start(out=w[:, :, 0:512], in_=r[:, :, 0:512])
                nc.sync.dma_start(out=w[:, :, 512:1024], in_=r[:, :, 512:1024])
                return w

            wr_sb = ldw(pl, "wr", wrT_d)
            wk_sb = ldw(pl, "wk", wkT_d)
            wv_sb = ldw(pl, "wv", wvT_d)
            sh_sb = ldw(pl, "sh", shT_d)
            wo_sb = ldw(pl, "wo", woT_d)

            hn_t = {}

            def stage_A1(j):
                """ln1 + PE-transpose -> hnc[j]"""
                hnc = pl.tile([P, CT, 513], BF16, tag="hnc", bufs=2, name=f"hnc{j}")
                hn_t[j] = hnc
                if j == 0:
                    nc.vector.memset(hnc[:, :, 0:1], 0.0)
                else:
                    nc.gpsimd.tensor_copy(hnc[:, :, 0:1], hn_t[j - 1][:, :, 512:513])
                for tl in range(4):
                    tt = 4 * j + tl
                    xt = pl.tile([P, C], F32, tag="xin", bufs=2, name=f"xt{tt}")
                    nc.sync.dma_start(out=xt, in_=x_d[tt * P:(tt + 1) * P, :])
                    st = ptl.tile([P, 2, 6], F32, tag="st", bufs=2, name=f"st{tt}")
                    nc.vector.bn_stats(out=st[:, 0, :], in_=xt[:, 0:512])
                    nc.vector.bn_stats(out=st[:, 1, :], in_=xt[:, 512:1024])
                    mv = ptl.tile([P, 2], F32, tag="mv", bufs=2, name=f"mv{tt}")
                    nc.vector.bn_aggr(out=mv, in_=st)
                    nc.gpsimd.tensor_copy(mus[:, 0, tt:tt + 1], mv[:, 0:1])
                    lv = ptl.tile([P, 1], F32, tag="lv", bufs=2, name=f"lv{tt}")
                    nc.scalar.activation(lv, mv[:, 1:2], ACT.Ln, bias=epst)
                    rstd = ptl.tile([P, 1], F32, tag="rstd", bufs=2, name=f"rstd{tt}")
                    nc.scalar.activation(rstd, lv, ACT.Exp, scale=-0.5)
                    nc.scalar.activation(mus[:, 1, tt:tt + 1], lv, ACT.Exp, scale=0.5)
                    negb = ptl.tile([P, 1], F32, tag="negb", bufs=2, name=f"negb{tt}")
                    nc.vector.tensor_scalar(negb, mv[:, 0:1], rstd, -1.0, AL.mult, AL.mult)
                    hnb = ptl.tile([P, C], BF16, tag="hnn", bufs=2, name=f"hnb{tt}")
                    nc.scalar.activation(hnb, xt, ACT.Identity, bias=negb, scale=rstd)
                    ptr = pp.tile([P, CT, P], BF16, tag="pt", name=f"ptr{tt}")
                    for ci in range(CT):
                        nc.tensor.transpose(ptr[:, ci, :], hnb[:, ci * P:(ci + 1) * P], ident)
                    nc.scalar.activation(hnc[:, :, 1 + tl * P:1 + (tl + 1) * P], ptr, ACT.Copy)

            def stage_MIX(j):
                """time-mixes for chunk j (hoisted one iteration early).
                xr/xk on DVE stt; xv decomposed on Pool (tt only)."""
                hnc = hn_t[j]
                xk = pl.tile([P, CT, 512], BF16, tag="xk", bufs=1, name=f"xk{j}")
                xv = pl.tile([P, CT, 512], BF16, tag="xv", bufs=1, name=f"xv{j}")
                xr = pl.tile([P, CT, 512], BF16, tag="xr", bufs=1, name=f"xr{j}")
                for ci in range(CT):
                    d = ptl.tile([P, 512], BF16, tag="mixd", bufs=2, name=f"d{j}_{ci}")
                    nc.gpsimd.tensor_sub(d, hnc[:, ci, 1:513], hnc[:, ci, 0:512])
                    nc.vector.scalar_tensor_tensor(xr[:, ci, :], d, cvc(S_MR, ci), hnc[:, ci, 0:512], AL.mult, AL.add)
                    nc.vector.scalar_tensor_tensor(xk[:, ci, :], d, cvc(S_MK, ci), hnc[:, ci, 0:512], AL.mult, AL.add)
                    nc.gpsimd.tensor_mul(xv[:, ci, :], d, _bcast_free(cvc(S_MV, ci), 512))
                    nc.gpsimd.tensor_add(xv[:, ci, :], xv[:, ci, :], hnc[:, ci, 0:512])
                return xk, xv, xr

            def stage_B1a(j, xr):
                """r matmuls + sigmoid batch"""
                sra = pl.tile([P, CT, 512], BF16, tag="sr", bufs=1, name=f"sra{j}")
                for co in range(CT):
                    pr = pp.tile([P, 512], F32, tag="p2", name=f"pr{j}_{co}")
                    for ci in range(CT):
                        nc.tensor.matmul(pr, wr_sb[:, ci, co * P:(co + 1) * P], xr[:, ci, :],
                                         start=(ci == 0), stop=(ci == CT - 1))
                    if j == 0:
                        nc.vector.tensor_scalar_add(pr[:, 0:1], pr[:, 0:1], cvc(S_FR, co))
                    nc.scalar.activation(sra[:, co, :], pr, ACT.Sigmoid, bias=cvc(S_RB, co))
                return sra

            def stage_B1b(j):
                """short matmuls + o1 = ps*std on Act"""
                hnc = hn_t[j]
                o1 = ptl.tile([P, 4, C], F32, tag="o1", bufs=1, name=f"o1_{j}")
                for tl in range(4):
                    tt = 4 * j + tl
                    for nco in range(NC2):
                        ps = pp.tile([P, 512], F32, tag="p0", name=f"ps{tt}_{nco}")
                        for ci in range(CT):
                            nc.tensor.matmul(ps, hnc[:, ci, 1 + tl * P:1 + (tl + 1) * P],
                                             sh_sb[:, ci, nco * 512:(nco + 1) * 512],
                                             start=(ci == 0), stop=(ci == CT - 1))
                        nc.scalar.activation(o1[:, tl, nco * 512:(nco + 1) * 512], ps,
                                             ACT.Identity, scale=mus[:, 1, tt:tt + 1])
                return o1

            def stage_B1c(j, xk, xv, sra):
                """k/v matmuls + Exp batch + WKV recurrence"""
                rw = pl.tile([P, CT, 512], BF16, tag="rw", bufs=1, name=f"rw{j}")
                for co in range(CT):
                    pk = pp.tile([P, 512], F32, tag="p2", name=f"pk{j}_{co}")
                    pv = pp.tile([P, 512], F32, tag="p1", name=f"pv{j}_{co}")
                    for ci in range(CT):
                        nc.tensor.matmul(pk, wk_sb[:, ci, co * P:(co + 1) * P], xk[:, ci, :],
                                         start=(ci == 0), stop=(ci == CT - 1))
                    for ci in range(CT):
                        nc.tensor.matmul(pv, wv_sb[:, ci, co * P:(co + 1) * P], xv[:, ci, :],
                                         start=(ci == 0), stop=(ci == CT - 1))
                    if j == 0:
                        nc.vector.tensor_scalar_add(pk[:, 0:1], pk[:, 0:1], cvc(S_FK, co))
                        nc.vector.tensor_scalar_add(pv[:, 0:1], pv[:, 0:1], cvc(S_FV, co))
                    ek = pl.tile([P, 512], F32, tag="ek", bufs=2, name=f"ek{j}_{co}")
                    nc.scalar.activation(ek, pk, ACT.Exp, bias=cvc(S_KB, co))
                    ekv = pl.tile([P, 512], F32, tag="ekv", bufs=2, name=f"ekv{j}_{co}")
                    nc.vector.scalar_tensor_tensor(ekv, pv, cvc(S_VB, co), ek, AL.add, AL.mult)
                    a_ = pl.tile([P, 513], F32, tag="a", bufs=1, name=f"a{j}_{co}")
                    b_ = pl.tile([P, 513], F32, tag="b", bufs=1, name=f"b{j}_{co}")
                    if j == 0:
                        nc.vector.memset(a_[:, 0:1], 0.0)
                        nc.vector.memset(b_[:, 0:1], 0.0)
                    else:
                        nc.vector.tensor_copy(a_[:, 0:1], carA[:, co:co + 1])
                        nc.vector.tensor_copy(b_[:, 0:1], carB[:, co:co + 1])
                    lam_bc = _bcast_free(cvc(S_LAM, co), 512)
                    nc.vector.tensor_tensor_scan(a_[:, 1:513], lam_bc, ekv, a_[:, 0:1], AL.mult, AL.add)
                    nc.vector.tensor_tensor_scan(b_[:, 1:513], lam_bc, ek, b_[:, 0:1], AL.mult, AL.add)
                    nc.gpsimd.tensor_copy(carA[:, co:co + 1], a_[:, 512:513])
                    nc.gpsimd.tensor_copy(carB[:, co:co + 1], b_[:, 512:513])
                    # num in-place on ekv, den in-place on ek (both DVE; short chain)
                    nc.vector.scalar_tensor_tensor(ekv, ekv, cvc(S_EU, co), a_[:, 0:512], AL.mult, AL.add)
                    nc.vector.scalar_tensor_tensor(ek, ek, cvc(S_EU, co), b_[:, 0:512], AL.mult, AL.add)
                    nc.vector.reciprocal(ek, ek)
                    nc.gpsimd.tensor_mul(ekv, ekv, ek)
                    nc.gpsimd.tensor_mul(rw[:, co, :], ekv, sra[:, co, :])
                return rw

            def stage_A2a(j, rw, o1):
                """wo matmuls + o1 assembly + spill"""
                for tl in range(4):
                    tt = 4 * j + tl
                    for nco in range(NC2):
                        pw = pp.tile([P, 512], F32, tag="p0", name=f"pw{tt}_{nco}")
                        for ci in range(CT):
                            nc.tensor.matmul(pw, rw[:, ci, tl * P:(tl + 1) * P],
                                             wo_sb[:, ci, nco * 512:(nco + 1) * 512],
                                             start=(ci == 0), stop=(ci == CT - 1))
                        o1s = o1[:, tl, nco * 512:(nco + 1) * 512]
                        nc.vector.tensor_add(o1s, o1s, pw)
                        nc.vector.scalar_tensor_tensor(o1s, srow_bc[:, nco * 512:(nco + 1) * 512],
                                                       mus[:, 0, tt:tt + 1], o1s, AL.mult, AL.add)
                    nc.sync.dma_start(out=o1d[tt * P:(tt + 1) * P, :], in_=o1[:, tl, :])

            def stage_A2b(j, o1):
                """ln2 + gn transpose + ffn mixes + spills (deferred one iter)"""
                gnc = ptl.tile([P, CT, 513], BF16, tag="gnc", bufs=1, name=f"gnc{j}")
                if j == 0:
                    nc.vector.memset(gnc[:, :, 0:1], 0.0)
                else:
                    nc.gpsimd.tensor_copy(gnc[:, :, 0:1], gcar[:, :, :])
                for tl in range(4):
                    tt = 4 * j + tl
                    st2 = ptl.tile([P, 2, 6], F32, tag="st", bufs=2, name=f"st2_{tt}")
                    nc.vector.bn_stats(out=st2[:, 0, :], in_=o1[:, tl, 0:512])
                    nc.vector.bn_stats(out=st2[:, 1, :], in_=o1[:, tl, 512:1024])
                    mv2 = ptl.tile([P, 2], F32, tag="mv", bufs=2, name=f"mv2_{tt}")
                    nc.vector.bn_aggr(out=mv2, in_=st2)
                    lv2 = ptl.tile([P, 1], F32, tag="lv", bufs=2, name=f"lv2_{tt}")
                    nc.scalar.activation(lv2, mv2[:, 1:2], ACT.Ln, bias=epst)
                    rstd2 = ptl.tile([P, 1], F32, tag="rstd", bufs=2, name=f"rstd2_{tt}")
                    nc.scalar.activation(rstd2, lv2, ACT.Exp, scale=-0.5)
                    negb2 = ptl.tile([P, 1], F32, tag="negb", bufs=2, name=f"negb2_{tt}")
                    nc.vector.tensor_scalar(negb2, mv2[:, 0:1], rstd2, -1.0, AL.mult, AL.mult)
                    gnb = ptl.tile([P, C], BF16, tag="hnn", bufs=2, name=f"gnb{tt}")
                    nc.scalar.activation(gnb, o1[:, tl, :], ACT.Identity, bias=negb2, scale=rstd2)
                    ptr2 = pp.tile([P, CT, P], BF16, tag="pt", name=f"ptr2_{tt}")
                    for ci in range(CT):
                        nc.tensor.transpose(ptr2[:, ci, :], gnb[:, ci * P:(ci + 1) * P], ident)
                    nc.scalar.activation(gnc[:, :, 1 + tl * P:1 + (tl + 1) * P], ptr2, ACT.Copy)
                nc.gpsimd.tensor_copy(gcar[:, :, :], gnc[:, :, 512:513])
                gk = ptl.tile([P, CT, 512], BF16, tag="gk", bufs=1, name=f"gk{j}")
                gr = ptl.tile([P, CT, 512], BF16, tag="gr", bufs=1, name=f"gr{j}")
                for ci in range(CT):
                    d2 = ptl.tile([P, 512], BF16, tag="mixd", bufs=2, name=f"d2_{j}_{ci}")
                    nc.gpsimd.tensor_sub(d2, gnc[:, ci, 1:513], gnc[:, ci, 0:512])
                    nc.vector.scalar_tensor_tensor(gk[:, ci, :], d2, cvc(S_FFK, ci), gnc[:, ci, 0:512], AL.mult, AL.add)
                    nc.vector.scalar_tensor_tensor(gr[:, ci, :], d2, cvc(S_FFR, ci), gnc[:, ci, 0:512], AL.mult, AL.add)
                nc.sync.dma_start(
                    out=gkTd[:, :].rearrange("(ci p) t -> p ci t", p=P)[:, :, j * 512:(j + 1) * 512],
                    in_=gk)
                nc.sync.dma_start(
                    out=grTd[:, :].rearrange("(ci p) t -> p ci t", p=P)[:, :, j * 512:(j + 1) * 512],
                    in_=gr)

            # ---------------- main loop ----------------
            stage_A1(0)
            mixes = stage_MIX(0)
            o1_t = {}
            for j in range(NJ):
                xk, xv, xr = mixes
                sra = stage_B1a(j, xr)
                if j >= 1:
                    stage_A2b(j - 1, o1_t.pop(j - 1))
                o1 = stage_B1b(j)
                o1_t[j] = o1
                rw = stage_B1c(j, xk, xv, sra)
                if j + 1 < NJ:
                    stage_A1(j + 1)
                    mixes = stage_MIX(j + 1)
                stage_A2a(j, rw, o1)

            pb_pool.release()

            # phase-3 front weights load while the last A2b tail runs
            p3a = tc.alloc_tile_pool(name="p3a", bufs=1, side="right")
            fwr_sb = ldw(p3a, "fwr", fwrT_d)
            fwk_sb = p3a.tile([P, CT, F], BF16, tag="fwk")
            rk = fwkT_d[:, :].rearrange("(ci p) fo -> p ci fo", p=P)
            for q in range(4):
                nc.sync.dma_start(out=fwk_sb[:, :, q * 1024:(q + 1) * 1024],
                                  in_=rk[:, :, q * 1024:(q + 1) * 1024])
            stage_A2b(NJ - 1, o1_t.pop(NJ - 1))
            pt_pool.release()

            p3b = tc.alloc_tile_pool(name="p3b", bufs=1, side="right")
            fwv_sb = p3b.tile([P, FT, C], BF16, tag="fwv")
            rv = fwvT_d[:, :].rearrange("(fi p) co -> p fi co", p=P)
            for q in range(4):
                nc.sync.dma_start(out=fwv_sb[:, q * 8:(q + 1) * 8, :],
                                  in_=rv[:, q * 8:(q + 1) * 8, :])

            # ---------------- phase 3: FFN back ----------------
            for j in range(NJ):
                gki = p3a.tile([P, CT, 512], BF16, tag="gki", bufs=1, name=f"gki{j}")
                nc.sync.dma_start(
                    in_=gkTd[:, :].rearrange("(ci p) t -> p ci t", p=P)[:, :, j * 512:(j + 1) * 512],
                    out=gki)
                gri = p3a.tile([P, CT, 512], BF16, tag="gri", bufs=1, name=f"gri{j}")
                nc.sync.dma_start(
                    in_=grTd[:, :].rearrange("(ci p) t -> p ci t", p=P)[:, :, j * 512:(j + 1) * 512],
                    out=gri)
                # fwr matmul + sigmoid + transpose to [t,c]
                s2t = p3b.tile([P, 4, C], BF16, tag="s2t", bufs=1, name=f"s2t{j}")
                for co in range(CT):
                    prr = pp.tile([P, 512], F32, tag="p2", name=f"prr{j}_{co}")
                    for ci in range(CT):
                        nc.tensor.matmul(prr, fwr_sb[:, ci, co * P:(co + 1) * P], gri[:, ci, :],
                                         start=(ci == 0), stop=(ci == CT - 1))
                    if j == 0:
                        nc.vector.tensor_scalar_add(prr[:, 0:1], prr[:, 0:1], cvc(S_FRR, co))
                    sr2 = p3b.tile([P, 512], BF16, tag="sr2", bufs=2, name=f"sr2_{j}_{co}")
                    nc.scalar.activation(sr2, prr, ACT.Sigmoid, bias=cvc(S_RRB, co))
                    ptr3 = pp.tile([P, 4, P], BF16, tag="pt", name=f"ptr3_{j}_{co}")
                    for tl in range(4):
                        nc.tensor.transpose(ptr3[:, tl, :], sr2[:, tl * P:(tl + 1) * P], ident)
                    nc.scalar.activation(s2t[:, :, co * P:(co + 1) * P], ptr3, ACT.Copy)
                # kk2 + kv, in two t-halves of 256
                for h in range(2):
                    kk2 = p3b.tile([P, FT, 256], BF16, tag="kk2", bufs=1, name=f"kk2_{j}_{h}")
                    hs = slice(h * 256, (h + 1) * 256)
                    for ft in range(FT):
                        pkk = pp.tile([P, 256], F32, tag="p0", name=f"pkk{j}_{h}_{ft}")
                        for ci in range(CT):
                            nc.tensor.matmul(pkk, fwk_sb[:, ci, ft * P:(ft + 1) * P], gki[:, ci, hs],
                                             start=(ci == 0), stop=(ci == CT - 1))
                        if j == 0 and h == 0:
                            nc.vector.tensor_scalar_add(pkk[:, 0:1], pkk[:, 0:1], fv[:, 1, ft:ft + 1])
                        kr = p3b.tile([P, 256], BF16, tag="kr", bufs=2, name=f"kr{j}_{h}_{ft}")
                        nc.scalar.activation(kr, pkk, ACT.Relu, bias=fv[:, 0, ft:ft + 1])
                        nc.vector.tensor_mul(kk2[:, ft, :], kr, kr)
                    for tlh in range(2):
                        tl = h * 2 + tlh
                        tt = 4 * j + tl
                        o1r = p3b.tile([P, C], F32, tag="o1r", bufs=2, name=f"o1r{tt}")
                        nc.sync.dma_start(out=o1r, in_=o1d[tt * P:(tt + 1) * P, :])
                        for nco in range(NC2):
                            pkv = pp.tile([P, 512], F32, tag="p1", name=f"pkv{tt}_{nco}")
                            for ft in range(FT):
                                nc.tensor.matmul(pkv, kk2[:, ft, tlh * P:(tlh + 1) * P],
                                                 fwv_sb[:, ft, nco * 512:(nco + 1) * 512],
                                                 start=(ft == 0), stop=(ft == FT - 1))
                            tmpv = p3b.tile([P, 512], F32, tag="kvt", bufs=2, name=f"kvt{tt}_{nco}")
                            nc.vector.tensor_mul(tmpv, pkv, s2t[:, tl, nco * 512:(nco + 1) * 512])
                            nc.vector.tensor_add(o1r[:, nco * 512:(nco + 1) * 512],
                                                 o1r[:, nco * 512:(nco + 1) * 512], tmpv)
                        nc.sync.dma_start(out=out_d[tt * P:(tt + 1) * P, :], in_=o1r)

            p3b.release()
            p3a.release()

    nc.compile()
    return nc


_NC_CACHE = {}


def get_nc(T):
    if T not in _NC_CACHE:
        _NC_CACHE[T] = build_nc(T)
    return _NC_CACHE[T]


def host_prep(inp, T):
    """Build per-core in_maps from full inputs (float64 math on host)."""
    f8 = lambda a: np.asarray(a, np.float64)
    x = np.asarray(inp["x"], np.float32)
    w1, b1 = f8(inp["ln1_w"]), f8(inp["ln1_b"])
    w2, b2 = f8(inp["ln2_w"]), f8(inp["ln2_b"])
    Wk, Wv, Wr, Wo = f8(inp["att_Wk"]), f8(inp["att_Wv"]), f8(inp["att_Wr"]), f8(inp["att_Wo"])
    Wsh = f8(inp["short_W"])
    fWk, fWr, fWv = f8(inp["ffn_Wk"]), f8(inp["ffn_Wr"]), f8(inp["ffn_Wv"])
    mk, mvx, mr = f8(inp["att_mix_k"]), f8(inp["att_mix_v"]), f8(inp["att_mix_r"])
    fk, fr = f8(inp["ffn_mix_k"]), f8(inp["ffn_mix_r"])
    decay, first = f8(inp["att_time_decay"]), f8(inp["att_time_first"])

    def pack_c(v):
        return np.asarray(v, np.float32).reshape(CT, P).T  # [128, CT]

    lam = np.exp(-np.exp(decay))
    eu = np.exp(first)
    kbias = Wk @ b1
    vbias = Wv @ b1
    rbias = Wr @ b1
    fixk = -Wk @ ((1.0 - mk) * b1)
    fixv = -Wv @ ((1.0 - mvx) * b1)
    fixr = -Wr @ ((1.0 - mr) * b1)
    kkbias = fWk @ b2
    fixkk = -fWk @ ((1.0 - fk) * b2)
    rrbias = fWr @ b2
    fixrr = -fWr @ ((1.0 - fr) * b2)
    srow = Wsh.sum(axis=1)

    cvec = np.stack([pack_c(v) for v in
                     [lam, eu, mk, mvx, mr, kbias, vbias, rbias,
                      fixk, fixv, fixr, fk, fr, rrbias, fixrr]], axis=1)  # [128, 15, 8]
    fvec = np.stack([np.asarray(v, np.float32).reshape(FT, P).T for v in [kkbias, fixkk]],
                    axis=1)  # [128, 2, 32]

    shared = {
        "wkT": np.ascontiguousarray((Wk * w1[None, :]).T.astype(BF)),
        "wvT": np.ascontiguousarray((Wv * w1[None, :]).T.astype(BF)),
        "wrT": np.ascontiguousarray((Wr * w1[None, :]).T.astype(BF)),
        "woT": np.ascontiguousarray(Wo.T.astype(BF)),
        "shT": np.ascontiguousarray(Wsh.T.astype(BF)),
        "fwkT": np.ascontiguousarray((fWk * w2[None, :]).T.astype(BF)),
        "fwrT": np.ascontiguousarray((fWr * w2[None, :]).T.astype(BF)),
        "fwvT": np.ascontiguousarray(fWv.T.astype(BF)),
        "cvec": np.ascontiguousarray(cvec.astype(np.float32)),
        "fvec": np.ascontiguousarray(fvec.astype(np.float32)),
        "srow": np.ascontiguousarray(srow.reshape(1, C).astype(BF)),
        "ident": np.ascontiguousarray(np.eye(P, dtype=np.float32).astype(BF)),
    }
    in_maps = []
    for b in range(x.shape[0]):
        m = dict(shared)
        m["x"] = np.ascontiguousarray(x[b, :T, :])
        in_maps.append(m)
    return in_maps


def kernel(**inputs):
    T = 2048
    nc = get_nc(T)
    in_maps = host_prep(inputs, T)
    res = run_bass_kernel_spmd(nc, in_maps, core_ids=list(range(len(in_maps))))
    out = np.stack([r["out"] for r in res.results], axis=0)
    return out.astype(np.float32)


# revision 29
# speedup vs baseline: 1.4727x; 1.0512x over previous
"""RWKV-style Block kernel for 8 Trainium2 NeuronCores (batch-parallel SPMD).

v4: software-pipelined main loop with DEFERRED sh/wo matmuls (one iteration
late, so the PE never waits on the current chunk's WKV recurrence), host-side
LN1 statistics (device does only the affine normalize), the short-cut mean
term folded into the matmul as a rank-1 update, PE transposes, batched Act
tables, staged SBUF pools so phase-3 weights load during the last chunk tail.

Iteration j emission (chunks of 512 tokens):
  A1(j+1):  x bf16 -> hn = (x-mu)*rstd (Act, host stats) -> PE-transpose
  MIX(j+1): time-mixes xk/xv/xr
  B1a(j):   r matmuls -> sigmoid batch
  B1c(j):   k/v matmuls -> Exp batch -> WKV scan -> rw
  A2a(j-1): short matmuls (+rank-1 mu/std*srow) -> o1=ps*std; wo matmuls,
            o1 += att; spill o1
  A2b(j-1): ln2 -> gn -> PE-transpose -> ffn mixes -> spill gkT/grT
Phase 3 per chunk: fwr matmul -> sigmoid sr2 -> PE-transpose [t,c];
  kk2 = relu(fwk@gk+b)^2 -> kv matmuls -> out = o1 + sr2*kv
"""
import numpy as np
import ml_dtypes

import concourse.bass as bass
import concourse.bacc as bacc
import concourse.mybir as mybir
import concourse.tile as tile
from concourse.bass_utils import run_bass_kernel_spmd

F32 = mybir.dt.float32
BF16 = mybir.dt.bfloat16
AL = mybir.AluOpType
ACT = mybir.ActivationFunctionType
BF = ml_dtypes.bfloat16

B, C, F = 8, 1024, 4096
P = 128
CT = C // P          # 8 c-tiles
FT = F // P          # 32 f-tiles
NC2 = C // 512       # 2
EPS = 1e-5

S_LAM, S_EU, S_MK, S_MV, S_MR, S_KB, S_VB, S_RB, S_FK, S_FV, S_FR, S_FFK, S_FFR, S_RRB, S_FRR = range(15)
NSLOT = 15
# lncol slots: rstd, negb=-mu*rstd, std
L_RSTD, L_NEGB, L_STD = range(3)


def _bcast_free(col_ap, n):
    """per-partition [128,1] column AP -> [128,n] stride-0 broadcast AP."""
    return bass.AP(tensor=col_ap.tensor, offset=col_ap.offset,
                   ap=[col_ap.ap[0], [0, n]])


def build_nc(T):
    NJ = T // 512
    TT = T // 128
    nc = bacc.Bacc("TRN2", target_bir_lowering=False)

    # ---------------- DRAM I/O ----------------
    x_d = nc.dram_tensor("x", [T, C], BF16, kind="ExternalInput")
    wkT_d = nc.dram_tensor("wkT", [C, C], BF16, kind="ExternalInput")
    wvT_d = nc.dram_tensor("wvT", [C, C], BF16, kind="ExternalInput")
    wrT_d = nc.dram_tensor("wrT", [C, C], BF16, kind="ExternalInput")
    woT_d = nc.dram_tensor("woT", [C, C], BF16, kind="ExternalInput")
    shT_d = nc.dram_tensor("shT", [C, C], BF16, kind="ExternalInput")
    fwkT_d = nc.dram_tensor("fwkT", [C, F], BF16, kind="ExternalInput")
    fwrT_d = nc.dram_tensor("fwrT", [C, C], BF16, kind="ExternalInput")
    fwvT_d = nc.dram_tensor("fwvT", [F, C], BF16, kind="ExternalInput")
    cvec_d = nc.dram_tensor("cvec", [P, NSLOT, CT], F32, kind="ExternalInput")
    fvec_d = nc.dram_tensor("fvec", [P, 2, FT], F32, kind="ExternalInput")
    lncol_d = nc.dram_tensor("lncol", [P, 3, TT], F32, kind="ExternalInput")
    musrow_d = nc.dram_tensor("musrow", [1, T], BF16, kind="ExternalInput")
    srow_d = nc.dram_tensor("srow", [1, C], BF16, kind="ExternalInput")
    ident_d = nc.dram_tensor("ident", [P, P], BF16, kind="ExternalInput")
    out_d = nc.dram_tensor("out", [T, C], F32, kind="ExternalOutput")

    # DRAM scratch (spills for phase 3)
    o1d = nc.dram_tensor("o1d", [T, C], BF16)
    gkTd = nc.dram_tensor("gkTd", [C, T], BF16)
    grTd = nc.dram_tensor("grTd", [C, T], BF16)

    with tile.TileContext(nc) as tc:
        with tc.tile_pool(name="const", bufs=1) as pc, \
             tc.tile_pool(name="psum", bufs=2, space="PSUM") as pp:

            # ---- constants / carries (outer pool, live whole kernel) ----
            cv = pc.tile([P, NSLOT, CT], F32, tag="cv")
            nc.sync.dma_start(out=cv, in_=cvec_d[:, :, :])
            fv = pc.tile([P, 2, FT], F32, tag="fv")
            nc.sync.dma_start(out=fv, in_=fvec_d[:, :, :])
            lncol = pc.tile([P, 3, TT], F32, tag="lncol")
            nc.sync.dma_start(out=lncol, in_=lncol_d[:, :, :])
            musrow = pc.tile([1, T], BF16, tag="musrow")
            nc.sync.dma_start(out=musrow, in_=musrow_d[:, :])
            srow = pc.tile([1, C], BF16, tag="srow")
            nc.sync.dma_start(out=srow, in_=srow_d[:, :])
            ident = pc.tile([P, P], BF16, tag="ident")
            nc.sync.dma_start(out=ident, in_=ident_d[:, :])
            epst = pc.tile([P, 1], F32, tag="epst")
            nc.vector.memset(epst, EPS)
            carA = pc.tile([P, CT], F32, tag="carA")
            carB = pc.tile([P, CT], F32, tag="carB")
            gcar = pc.tile([P, CT, 1], BF16, tag="gcar")

            def cvc(slot, ci):
                return cv[:, slot, ci:ci + 1]

            def lnc(slot, tt):
                return lncol[:, slot, tt:tt + 1]

            # tail pool outlives the big pool (deferred A2b overlaps phase-3
            # weight loads); phase-3 pools live on the right side of SBUF.
            pt_pool = tc.alloc_tile_pool(name="tail", bufs=1)
            pb_pool = tc.alloc_tile_pool(name="big", bufs=1)
            pl = pb_pool
            ptl = pt_pool

            def ldw(pool, tag, dram):
                w = pool.tile([P, CT, C], BF16, tag=tag)
                r = dram[:, :].rearrange("(ci p) co -> p ci co", p=P)
                nc.sync.dma_start(out=w[:, :, 0:512], in_=r[:, :, 0:512])
                nc.sync.dma_start(out=w[:, :, 512:1024], in_=r[:, :, 512:1024])
                return w

            hn_t = {}
            w_sb = {}

            def stage_A1(j, xpre=None):
                """hn = (x-mu)*rstd via Act (host stats) + PE-transpose"""
                hnc = ptl.tile([P, CT, 513], BF16, tag="hnc", bufs=3, name=f"hnc{j}")
                hn_t[j] = hnc
                if j == 0:
                    nc.vector.memset(hnc[:, :, 0:1], 0.0)
                else:
                    nc.gpsimd.tensor_copy(hnc[:, :, 0:1], hn_t[j - 1][:, :, 512:513])
                for tl in range(4):
                    tt = 4 * j + tl
                    if xpre is not None:
                        xt = xpre[:, tl, :]
                    else:
                        xt = pl.tile([P, C], BF16, tag="xin", bufs=2, name=f"xt{tt}")
                        nc.sync.dma_start(out=xt, in_=x_d[tt * P:(tt + 1) * P, :])
                    hnb = ptl.tile([P, C], BF16, tag="hnn", bufs=2, name=f"hnb{tt}")
                    nc.scalar.activation(hnb, xt, ACT.Identity,
                                         bias=lnc(L_NEGB, tt), scale=lnc(L_RSTD, tt))
                    ptr = pp.tile([P, CT, P], BF16, tag="pt", name=f"ptr{tt}")
                    for ci in range(CT):
                        nc.tensor.transpose(ptr[:, ci, :], hnb[:, ci * P:(ci + 1) * P], ident)
                    nc.scalar.activation(hnc[:, :, 1 + tl * P:1 + (tl + 1) * P], ptr, ACT.Copy)

            def stage_MIX(j):
                """time-mixes: xr/xk on DVE stt; xv decomposed on Pool"""
                hnc = hn_t[j]
                xk = pl.tile([P, CT, 512], BF16, tag="xk", bufs=1, name=f"xk{j}")
                xv = pl.tile([P, CT, 512], BF16, tag="xv", bufs=1, name=f"xv{j}")
                xr = pl.tile([P, CT, 512], BF16, tag="xr", bufs=1, name=f"xr{j}")
                for ci in range(CT):
                    d = ptl.tile([P, 512], BF16, tag="mixd", bufs=2, name=f"d{j}_{ci}")
                    nc.gpsimd.tensor_sub(d, hnc[:, ci, 1:513], hnc[:, ci, 0:512])
                    nc.vector.scalar_tensor_tensor(xr[:, ci, :], d, cvc(S_MR, ci), hnc[:, ci, 0:512], AL.mult, AL.add)
                    nc.vector.scalar_tensor_tensor(xk[:, ci, :], d, cvc(S_MK, ci), hnc[:, ci, 0:512], AL.mult, AL.add)
                    nc.gpsimd.tensor_mul(xv[:, ci, :], d, _bcast_free(cvc(S_MV, ci), 512))
                    nc.gpsimd.tensor_add(xv[:, ci, :], xv[:, ci, :], hnc[:, ci, 0:512])
                return xk, xv, xr

            def stage_B1a(j, xr):
                """r matmuls + sigmoid batch"""
                sra = pl.tile([P, CT, 512], BF16, tag="sr", bufs=1, name=f"sra{j}")
                for co in range(CT):
                    pr = pp.tile([P, 512], F32, tag="p2", name=f"pr{j}_{co}")
                    for ci in range(CT):
                        nc.tensor.matmul(pr, wr_sb[:, ci, co * P:(co + 1) * P], xr[:, ci, :],
                                         start=(ci == 0), stop=(ci == CT - 1))
                    if j == 0:
                        nc.vector.tensor_scalar_add(pr[:, 0:1], pr[:, 0:1], cvc(S_FR, co))
                    nc.scalar.activation(sra[:, co, :], pr, ACT.Sigmoid, bias=cvc(S_RB, co))
                return sra

            def stage_B1c(j, xk, xv, sra, fillers=()):
                """k/v matmuls + Exp batch + WKV recurrence; fillers are
                PE work (deferred short-matmul groups) interleaved per co
                to pace k/v production to the DVE consumption rate."""
                rw = ptl.tile([P, CT, 512], BF16, tag="rw", bufs=1, name=f"rw{j}")
                for co in range(CT):
                    if co < len(fillers):
                        fillers[co]()
                    pk = pp.tile([P, 512], F32, tag="p2", name=f"pk{j}_{co}")
                    pv = pp.tile([P, 512], F32, tag="p1", name=f"pv{j}_{co}")
                    for ci in range(CT):
                        nc.tensor.matmul(pk, wk_sb[:, ci, co * P:(co + 1) * P], xk[:, ci, :],
                                         start=(ci == 0), stop=(ci == CT - 1))
                    for ci in range(CT):
                        nc.tensor.matmul(pv, wv_sb[:, ci, co * P:(co + 1) * P], xv[:, ci, :],
                                         start=(ci == 0), stop=(ci == CT - 1))
                    if j == 0:
                        nc.vector.tensor_scalar_add(pk[:, 0:1], pk[:, 0:1], cvc(S_FK, co))
                        nc.vector.tensor_scalar_add(pv[:, 0:1], pv[:, 0:1], cvc(S_FV, co))
                    ek = pl.tile([P, 512], F32, tag="ek", bufs=2, name=f"ek{j}_{co}")
                    nc.scalar.activation(ek, pk, ACT.Exp, bias=cvc(S_KB, co))
                    ekv = pl.tile([P, 512], F32, tag="ekv", bufs=1, name=f"ekv{j}_{co}")
                    nc.vector.scalar_tensor_tensor(ekv, pv, cvc(S_VB, co), ek, AL.add, AL.mult)
                    a_ = pl.tile([P, 513], F32, tag="a", bufs=1, name=f"a{j}_{co}")
                    b_ = pl.tile([P, 513], F32, tag="b", bufs=1, name=f"b{j}_{co}")
                    if j == 0:
                        nc.vector.memset(a_[:, 0:1], 0.0)
                        nc.vector.memset(b_[:, 0:1], 0.0)
                    else:
                        nc.vector.tensor_copy(a_[:, 0:1], carA[:, co:co + 1])
                        nc.vector.tensor_copy(b_[:, 0:1], carB[:, co:co + 1])
                    lam_bc = _bcast_free(cvc(S_LAM, co), 512)
                    nc.vector.tensor_tensor_scan(a_[:, 1:513], lam_bc, ekv, a_[:, 0:1], AL.mult, AL.add)
                    nc.vector.tensor_tensor_scan(b_[:, 1:513], lam_bc, ek, b_[:, 0:1], AL.mult, AL.add)
                    nc.gpsimd.tensor_copy(carA[:, co:co + 1], a_[:, 512:513])
                    nc.gpsimd.tensor_copy(carB[:, co:co + 1], b_[:, 512:513])
                    # num in-place on ekv, den in-place on ek (DVE); muls on Pool
                    nc.vector.scalar_tensor_tensor(ekv, ekv, cvc(S_EU, co), a_[:, 0:512], AL.mult, AL.add)
                    nc.vector.scalar_tensor_tensor(ek, ek, cvc(S_EU, co), b_[:, 0:512], AL.mult, AL.add)
                    nc.vector.reciprocal(ek, ek)
                    nc.gpsimd.tensor_mul(ekv, ekv, ek)
                    nc.gpsimd.tensor_mul(rw[:, co, :], ekv, sra[:, co, :])
                return rw

            def alloc_o1(j):
                return ptl.tile([P, 4, C], BF16, tag="o1", bufs=1, name=f"o1_{j}")

            def make_ps_fillers(j, o1):
                """short (+rank-1) matmul groups as per-co filler closures"""
                hnc = hn_t[j]
                def mk(tl, nco):
                    def emit():
                        tt = 4 * j + tl
                        ps = pp.tile([P, 512], F32, tag="p0", name=f"ps{tt}_{nco}")
                        for ci in range(CT):
                            nc.tensor.matmul(ps, hnc[:, ci, 1 + tl * P:1 + (tl + 1) * P],
                                             sh_sb[:, ci, nco * 512:(nco + 1) * 512],
                                             start=(ci == 0), stop=False)
                        # rank-1: (mu/std)_t * srow_co ; then o1 = psum*std
                        nc.tensor.matmul(ps, musrow[0:1, tt * P:(tt + 1) * P],
                                         srow[0:1, nco * 512:(nco + 1) * 512],
                                         start=False, stop=True)
                        nc.scalar.activation(o1[:, tl, nco * 512:(nco + 1) * 512], ps,
                                             ACT.Identity, scale=lnc(L_STD, tt))
                    return emit
                return [mk(tl, nco) for tl in range(4) for nco in range(NC2)]

            def stage_A2a_wo(j, rw, o1):
                """wo matmuls + o1 assembly + spill"""
                for tl in range(4):
                    tt = 4 * j + tl
                    for nco in range(NC2):
                        pw = pp.tile([P, 512], F32, tag="p0", name=f"pw{tt}_{nco}")
                        for ci in range(CT):
                            nc.tensor.matmul(pw, rw[:, ci, tl * P:(tl + 1) * P],
                                             wo_sb[:, ci, nco * 512:(nco + 1) * 512],
                                             start=(ci == 0), stop=(ci == CT - 1))
                        o1s = o1[:, tl, nco * 512:(nco + 1) * 512]
                        nc.vector.tensor_add(o1s, o1s, pw)
                    nc.sync.dma_start(out=o1d[tt * P:(tt + 1) * P, :], in_=o1[:, tl, :])

            def stage_A2b(j, o1):
                """ln2 + gn transpose + ffn mixes + spills"""
                gnc = ptl.tile([P, CT, 513], BF16, tag="gnc", bufs=1, name=f"gnc{j}")
                if j == 0:
                    nc.vector.memset(gnc[:, :, 0:1], 0.0)
                else:
                    nc.gpsimd.tensor_copy(gnc[:, :, 0:1], gcar[:, :, :])
                for tl in range(4):
                    tt = 4 * j + tl
                    st2 = ptl.tile([P, 2, 6], F32, tag="st", bufs=2, name=f"st2_{tt}")
                    nc.vector.bn_stats(out=st2[:, 0, :], in_=o1[:, tl, 0:512])
                    nc.vector.bn_stats(out=st2[:, 1, :], in_=o1[:, tl, 512:1024])
                    mv2 = ptl.tile([P, 2], F32, tag="mv", bufs=2, name=f"mv2_{tt}")
                    nc.vector.bn_aggr(out=mv2, in_=st2)
                    lv2 = ptl.tile([P, 1], F32, tag="lv", bufs=2, name=f"lv2_{tt}")
                    nc.scalar.activation(lv2, mv2[:, 1:2], ACT.Ln, bias=epst)
                    rstd2 = ptl.tile([P, 1], F32, tag="rstd", bufs=2, name=f"rstd2_{tt}")
                    nc.scalar.activation(rstd2, lv2, ACT.Exp, scale=-0.5)
                    negb2 = ptl.tile([P, 1], F32, tag="negb", bufs=2, name=f"negb2_{tt}")
                    nc.vector.tensor_scalar(negb2, mv2[:, 0:1], rstd2, -1.0, AL.mult, AL.mult)
                    gnb = ptl.tile([P, C], BF16, tag="hnn", bufs=2, name=f"gnb{tt}")
                    nc.scalar.activation(gnb, o1[:, tl, :], ACT.Identity, bias=negb2, scale=rstd2)
                    ptr2 = pp.tile([P, CT, P], BF16, tag="pt", name=f"ptr2_{tt}")
                    for ci in range(CT):
                        nc.tensor.transpose(ptr2[:, ci, :], gnb[:, ci * P:(ci + 1) * P], ident)
                    nc.scalar.activation(gnc[:, :, 1 + tl * P:1 + (tl + 1) * P], ptr2, ACT.Copy)
                nc.gpsimd.tensor_copy(gcar[:, :, :], gnc[:, :, 512:513])
                gk = ptl.tile([P, CT, 512], BF16, tag="gk", bufs=1, name=f"gk{j}")
                gr = ptl.tile([P, CT, 512], BF16, tag="gr", bufs=1, name=f"gr{j}")
                for ci in range(CT):
                    d2 = ptl.tile([P, 512], BF16, tag="mixd", bufs=2, name=f"d2_{j}_{ci}")
                    nc.gpsimd.tensor_sub(d2, gnc[:, ci, 1:513], gnc[:, ci, 0:512])
                    nc.vector.scalar_tensor_tensor(gk[:, ci, :], d2, cvc(S_FFK, ci), gnc[:, ci, 0:512], AL.mult, AL.add)
                    nc.vector.scalar_tensor_tensor(gr[:, ci, :], d2, cvc(S_FFR, ci), gnc[:, ci, 0:512], AL.mult, AL.add)
                nc.sync.dma_start(
                    out=gkTd[:, :].rearrange("(ci p) t -> p ci t", p=P)[:, :, j * 512:(j + 1) * 512],
                    in_=gk)
                nc.sync.dma_start(
                    out=grTd[:, :].rearrange("(ci p) t -> p ci t", p=P)[:, :, j * 512:(j + 1) * 512],
                    in_=gr)

            # ---------------- main loop ----------------
            stage_A1(0)
            mix_t = {0: stage_MIX(0)}
            wr_sb = w_sb["wr"] = ldw(pl, "wr", wrT_d)
            wk_sb = w_sb["wk"] = ldw(pl, "wk", wkT_d)
            wv_sb = w_sb["wv"] = ldw(pl, "wv", wvT_d)
            sh_sb = w_sb["sh"] = ldw(ptl, "sh", shT_d)
            wo_sb = w_sb["wo"] = ldw(ptl, "wo", woT_d)
            rw_t = {}
            for j in range(NJ):
                if j + 1 < NJ:
                    stage_A1(j + 1)
                xk, xv, xr = mix_t.pop(j)
                sra = stage_B1a(j, xr)
                if j >= 1:
                    o1p = alloc_o1(j - 1)
                    fillers = make_ps_fillers(j - 1, o1p)
                else:
                    o1p, fillers = None, ()
                rw_t[j] = stage_B1c(j, xk, xv, sra, fillers)
                if j + 1 < NJ:
                    mix_t[j + 1] = stage_MIX(j + 1)
                if j >= 1:
                    stage_A2a_wo(j - 1, rw_t.pop(j - 1), o1p)
                    stage_A2b(j - 1, o1p)

            pb_pool.release()

            # phase-3 front weights load while the last A2b tail runs
            p3a = tc.alloc_tile_pool(name="p3a", bufs=1, side="right")
            fwr_sb = ldw(p3a, "fwr", fwrT_d)
            fwk_sb = p3a.tile([P, CT, F], BF16, tag="fwk")
            rk = fwkT_d[:, :].rearrange("(ci p) fo -> p ci fo", p=P)
            for q in range(4):
                nc.sync.dma_start(out=fwk_sb[:, :, q * 1024:(q + 1) * 1024],
                                  in_=rk[:, :, q * 1024:(q + 1) * 1024])

            o1_last = alloc_o1(NJ - 1)
            for f in make_ps_fillers(NJ - 1, o1_last):
                f()
            stage_A2a_wo(NJ - 1, rw_t.pop(NJ - 1), o1_last)
            stage_A2b(NJ - 1, o1_last)
            pt_pool.release()

            p3b = tc.alloc_tile_pool(name="p3b", bufs=1, side="right")
            fwv_sb = p3b.tile([P, FT, C], BF16, tag="fwv")
            rv = fwvT_d[:, :].rearrange("(fi p) co -> p fi co", p=P)
            for q in range(4):
                nc.sync.dma_start(out=fwv_sb[:, q * 8:(q + 1) * 8, :],
                                  in_=rv[:, q * 8:(q + 1) * 8, :])

            # ---------------- phase 3: FFN back ----------------
            for j in range(NJ):
                gki = p3a.tile([P, CT, 512], BF16, tag="gki", bufs=1, name=f"gki{j}")
                nc.sync.dma_start(
                    in_=gkTd[:, :].rearrange("(ci p) t -> p ci t", p=P)[:, :, j * 512:(j + 1) * 512],
                    out=gki)
                gri = p3a.tile([P, CT, 512], BF16, tag="gri", bufs=1, name=f"gri{j}")
                nc.sync.dma_start(
                    in_=grTd[:, :].rearrange("(ci p) t -> p ci t", p=P)[:, :, j * 512:(j + 1) * 512],
                    out=gri)
                # fwr matmul + sigmoid + transpose to [t,c]
                s2t = p3b.tile([P, 4, C], BF16, tag="s2t", bufs=1, name=f"s2t{j}")
                for co in range(CT):
                    prr = pp.tile([P, 512], F32, tag="p2", name=f"prr{j}_{co}")
                    for ci in range(CT):
                        nc.tensor.matmul(prr, fwr_sb[:, ci, co * P:(co + 1) * P], gri[:, ci, :],
                                         start=(ci == 0), stop=(ci == CT - 1))
                    if j == 0:
                        nc.vector.tensor_scalar_add(prr[:, 0:1], prr[:, 0:1], cvc(S_FRR, co))
                    sr2 = p3b.tile([P, 512], BF16, tag="sr2", bufs=2, name=f"sr2_{j}_{co}")
                    nc.scalar.activation(sr2, prr, ACT.Sigmoid, bias=cvc(S_RRB, co))
                    ptr3 = pp.tile([P, 4, P], BF16, tag="pt", name=f"ptr3_{j}_{co}")
                    for tl in range(4):
                        nc.tensor.transpose(ptr3[:, tl, :], sr2[:, tl * P:(tl + 1) * P], ident)
                    nc.scalar.activation(s2t[:, :, co * P:(co + 1) * P], ptr3, ACT.Copy)
                # kk2 + kv, in two t-halves of 256
                for h in range(2):
                    kk2 = p3b.tile([P, FT, 256], BF16, tag="kk2", bufs=1, name=f"kk2_{j}_{h}")
                    hs = slice(h * 256, (h + 1) * 256)
                    for ft in range(FT):
                        pkk = pp.tile([P, 256], F32, tag="p0", name=f"pkk{j}_{h}_{ft}")
                        for ci in range(CT):
                            nc.tensor.matmul(pkk, fwk_sb[:, ci, ft * P:(ft + 1) * P], gki[:, ci, hs],
                                             start=(ci == 0), stop=(ci == CT - 1))
                        if j == 0 and h == 0:
                            nc.vector.tensor_scalar_add(pkk[:, 0:1], pkk[:, 0:1], fv[:, 1, ft:ft + 1])
                        kr = p3b.tile([P, 256], BF16, tag="kr", bufs=2, name=f"kr{j}_{h}_{ft}")
                        nc.scalar.activation(kr, pkk, ACT.Relu, bias=fv[:, 0, ft:ft + 1])
                        nc.vector.tensor_mul(kk2[:, ft, :], kr, kr)
                    for tlh in range(2):
                        tl = h * 2 + tlh
                        tt = 4 * j + tl
                        o1r = p3b.tile([P, C], BF16, tag="o1r", bufs=2, name=f"o1r{tt}")
                        nc.sync.dma_start(out=o1r, in_=o1d[tt * P:(tt + 1) * P, :])
                        for nco in range(NC2):
                            pkv = pp.tile([P, 512], F32, tag="p1", name=f"pkv{tt}_{nco}")
                            for ft in range(FT):
                                nc.tensor.matmul(pkv, kk2[:, ft, tlh * P:(tlh + 1) * P],
                                                 fwv_sb[:, ft, nco * 512:(nco + 1) * 512],
                                                 start=(ft == 0), stop=(ft == FT - 1))
                            tmpv = p3b.tile([P, 512], F32, tag="kvt", bufs=2, name=f"kvt{tt}_{nco}")
                            nc.vector.tensor_mul(tmpv, pkv, s2t[:, tl, nco * 512:(nco + 1) * 512])
                            nc.vector.tensor_add(tmpv, tmpv, o1r[:, nco * 512:(nco + 1) * 512])
                            nc.sync.dma_start(out=out_d[tt * P:(tt + 1) * P, nco * 512:(nco + 1) * 512],
                                              in_=tmpv)

            p3b.release()
            p3a.release()

    nc.compile()
    return nc


_NC_CACHE = {}


def get_nc(T):
    if T not in _NC_CACHE:
        _NC_CACHE[T] = build_nc(T)
    return _NC_CACHE[T]


def host_prep(inp, T):
    """Build per-core in_maps from full inputs (float64 math on host)."""
    f8 = lambda a: np.asarray(a, np.float64)
    x = np.asarray(inp["x"], np.float32)
    w1, b1 = f8(inp["ln1_w"]), f8(inp["ln1_b"])
    w2, b2 = f8(inp["ln2_w"]), f8(inp["ln2_b"])
    Wk, Wv, Wr, Wo = f8(inp["att_Wk"]), f8(inp["att_Wv"]), f8(inp["att_Wr"]), f8(inp["att_Wo"])
    Wsh = f8(inp["short_W"])
    fWk, fWr, fWv = f8(inp["ffn_Wk"]), f8(inp["ffn_Wr"]), f8(inp["ffn_Wv"])
    mk, mvx, mr = f8(inp["att_mix_k"]), f8(inp["att_mix_v"]), f8(inp["att_mix_r"])
    fk, fr = f8(inp["ffn_mix_k"]), f8(inp["ffn_mix_r"])
    decay, first = f8(inp["att_time_decay"]), f8(inp["att_time_first"])

    def pack_c(v):
        return np.asarray(v, np.float32).reshape(CT, P).T  # [128, CT]

    lam = np.exp(-np.exp(decay))
    eu = np.exp(first)
    kbias = Wk @ b1
    vbias = Wv @ b1
    rbias = Wr @ b1
    fixk = -Wk @ ((1.0 - mk) * b1)
    fixv = -Wv @ ((1.0 - mvx) * b1)
    fixr = -Wr @ ((1.0 - mr) * b1)
    kkbias = fWk @ b2
    fixkk = -fWk @ ((1.0 - fk) * b2)
    rrbias = fWr @ b2
    fixrr = -fWr @ ((1.0 - fr) * b2)
    srow = Wsh.sum(axis=1)

    cvec = np.stack([pack_c(v) for v in
                     [lam, eu, mk, mvx, mr, kbias, vbias, rbias,
                      fixk, fixv, fixr, fk, fr, rrbias, fixrr]], axis=1)  # [128, 15, 8]
    fvec = np.stack([np.asarray(v, np.float32).reshape(FT, P).T for v in [kkbias, fixkk]],
                    axis=1)  # [128, 2, 32]

    shared = {
        "wkT": np.ascontiguousarray((Wk * w1[None, :]).T.astype(BF)),
        "wvT": np.ascontiguousarray((Wv * w1[None, :]).T.astype(BF)),
        "wrT": np.ascontiguousarray((Wr * w1[None, :]).T.astype(BF)),
        "woT": np.ascontiguousarray(Wo.T.astype(BF)),
        "shT": np.ascontiguousarray(Wsh.T.astype(BF)),
        "fwkT": np.ascontiguousarray((fWk * w2[None, :]).T.astype(BF)),
        "fwrT": np.ascontiguousarray((fWr * w2[None, :]).T.astype(BF)),
        "fwvT": np.ascontiguousarray(fWv.T.astype(BF)),
        "cvec": np.ascontiguousarray(cvec.astype(np.float32)),
        "fvec": np.ascontiguousarray(fvec.astype(np.float32)),
        "srow": np.ascontiguousarray(srow.reshape(1, C).astype(BF)),
        "ident": np.ascontiguousarray(np.eye(P, dtype=np.float32).astype(BF)),
    }
    TTl = T // P
    in_maps = []
    for b in range(x.shape[0]):
        m = dict(shared)
        xb = f8(x[b, :T, :])
        mu = xb.mean(axis=1)
        var = xb.var(axis=1)
        std = np.sqrt(var + EPS)
        rstd = 1.0 / std
        negb = -mu * rstd
        lncol = np.stack([rstd, negb, std], axis=0)          # [3, T]
        lncol = lncol.reshape(3, TTl, P).transpose(2, 0, 1)  # [128, 3, TT]
        m["lncol"] = np.ascontiguousarray(lncol.astype(np.float32))
        m["musrow"] = np.ascontiguousarray((mu * rstd).reshape(1, T).astype(BF))
        m["x"] = np.ascontiguousarray(x[b, :T, :].astype(BF))
        in_maps.append(m)
    return in_maps


def kernel(**inputs):
    T = 2048
    nc = get_nc(T)
    in_maps = host_prep(inputs, T)
    res = run_bass_kernel_spmd(nc, in_maps, core_ids=list(range(len(in_maps))))
    out = np.stack([r["out"] for r in res.results], axis=0)
    return out.astype(np.float32)
